# revision 16
# baseline (speedup 1.0000x reference)
"""Tensor-parallel llama-style attention (prefill) on 8 TRN2 NeuronCores.

Sharding: tensor-parallel over heads. Core c holds q-heads [4c, 4c+4),
kv-head c, the matching rows of wq/wk/wv, and columns [512c, 512c+512) of
wo. Each core computes a full-size partial of the output projection;
partials are summed on the host (the "all-reduce after wo").

Device-side layout (causal fast path):
  - All matmul operands are bf16 (PSUM accumulation stays fp32). bf16
    enables the compiler's fast-weight-load path and avoids the fp32
    PE power-throttle (HAM drops the PE clock to 1.2 GHz under
    sustained fp32-mode matmul); tolerance is 2e-2, bf16 lands ~1e-3.
  - Activations kept transposed (feature dim on partitions): xT
    [DIM, TOK], Q^T/K^T [128, S] per head, V in token-major chunks.
  - wo ([128, 4, DIM] bf16, 32 KiB/partition) and Q^T (16 KiB/partition)
    are SBUF-resident: no DRAM spill, no W-phase weight streaming.
  - RoPE: head-dim basis permuted on the host (even first, odd second),
    turning the interleaved rotation into a half-partition swap +
    elementwise mul/add against cos/sin tables, read from PSUM.
  - Causal mask: one [128,128] triangle tile (the diagonal-block
    pattern only depends on k-q). Off-diagonal blocks below the
    diagonal need no mask; blocks above are never computed. Diagonal
    block j additionally restricts its q-range to [j*128, 512), which
    recovers 128-granular causal savings (136/160 of the block-level
    work) while keeping 512-wide moving operands.
  - Softmax: no max-subtraction (scores*scale is O(10); exp safe in
    fp32). Row sums via a ones-vector matmul on the tensor engine; the
    reciprocal is partition-broadcast by gpsimd and applied by DVE.
"""

import math
import os
import sys

sys.path.insert(0, "/opt/trn_rl_repo")

import numpy as np
import ml_dtypes

import concourse.bacc as bacc
import concourse.tile as tile
import concourse.mybir as mybir
from concourse import masks
from concourse.bass_utils import run_bass_kernel_spmd

B, S, DIM = 2, 2048, 4096
TOK = B * S
NH, NKV, HD = 32, 8, 128
NCORES = 8
HQ = NH // NCORES            # 4 query heads per core
SCALE = 1.0 / math.sqrt(HD)
F32 = mybir.dt.float32
F32R = mybir.dt.float32r
BF16 = mybir.dt.bfloat16
NPBF16 = ml_dtypes.bfloat16
EXP = mybir.ActivationFunctionType.Exp

QB = 4          # q-blocks per batch (512 queries each)
QW = S // QB    # 512
KT = S // 128   # 16 k-tiles per batch
NJ = HQ + 2     # 6 projection output tiles: 4 Q heads, K, V


def _build_causal():
    nc = bacc.Bacc("TRN2", target_bir_lowering=False, debug=False)

    xT_d = nc.dram_tensor("xT", [DIM, TOK], BF16, kind="ExternalInput")
    w_d = nc.dram_tensor("wqkvT", [DIM, NJ * HD], BF16, kind="ExternalInput")
    wo_d = nc.dram_tensor("woT", [HQ * HD, DIM], BF16, kind="ExternalInput")
    cos_d = nc.dram_tensor("cosT", [HD, S], F32, kind="ExternalInput")
    sin_d = nc.dram_tensor("sinTs", [HD, S], F32, kind="ExternalInput")
    tri_d = nc.dram_tensor("triM", [128, 128], F32, kind="ExternalInput")
    out_d = nc.dram_tensor("out_part", [TOK, DIM], F32, kind="ExternalOutput")

    xT = xT_d.ap().rearrange("(kt p) t -> p kt t", p=128)      # [128, 32, TOK]
    w_ap = w_d.ap().rearrange("(kt p) j -> p kt j", p=128)     # [128, 32, 768]
    wo_ap = wo_d.ap().rearrange("(dt p) m -> p dt m", p=128)   # [128, 4, DIM]
    out_v = out_d.ap().rearrange("(g p) m -> p g m", p=128)    # [128, 32, DIM]

    with tile.TileContext(nc) as tc:
        with (
            tc.tile_pool(name="const", bufs=1) as const_pool,
            tc.tile_pool(name="batch", bufs=1) as batch_pool,
        ):
            wqkv = const_pool.tile([128, 32, NJ * HD], BF16)
            kc0 = 0
            for kcn in (2, 2, 4, 4, 4, 4, 4, 4, 4):  # small first chunks so
                nc.scalar.dma_start(wqkv[:, kc0:kc0 + kcn, :],  # P1 starts early
                                    w_ap[:, kc0:kc0 + kcn, :])
                kc0 += kcn
            wo_all = const_pool.tile([128, HQ, DIM], BF16)
            for mc in range(4):
                nc.gpsimd.dma_start(wo_all[:, :, mc * 1024:(mc + 1) * 1024],
                                    wo_ap[:, :, mc * 1024:(mc + 1) * 1024])
            tri = const_pool.tile([128, 128], F32)
            nc.gpsimd.dma_start(tri[:], tri_d.ap())
            ident = const_pool.tile([128, 128], BF16)
            masks.make_identity(nc, ident[:])
            ones_col = const_pool.tile([128, 1], BF16)
            nc.vector.memset(ones_col[:], 1.0)

            # per-batch SBUF-resident tensors
            kT_s = batch_pool.tile([128, S], BF16)
            v_s = batch_pool.tile([128, KT, HD], BF16)
            att_h = batch_pool.tile([128, HQ, S], BF16)
            qT_s = batch_pool.tile([128, HQ, S], BF16)

            for b in range(B):
                # ---------- P1: QKV projections + RoPE ----------
                with (
                    tc.tile_pool(name="xt", bufs=2) as xt_pool,
                    tc.tile_pool(name="cs", bufs=2) as cs_pool,
                    tc.tile_pool(name="rope", bufs=2) as rope_pool,
                    tc.tile_pool(name="vtmp", bufs=2) as vtmp_pool,
                    tc.tile_pool(name="p1ps", bufs=NJ, space="PSUM") as p1ps,
                    tc.tile_pool(name="trps", bufs=2, space="PSUM") as trps,
                ):
                    for tb in range(4):          # 512-token chunks
                        c0 = b * S + tb * 512
                        sl = slice(tb * 512, tb * 512 + 512)
                        cos_c = cs_pool.tile([HD, 512], F32, tag="cos")
                        sin_c = cs_pool.tile([HD, 512], F32, tag="sin")
                        nc.gpsimd.dma_start(cos_c[:], cos_d.ap()[:, sl])
                        nc.gpsimd.dma_start(sin_c[:], sin_d.ap()[:, sl])
                        pss = [p1ps.tile([128, 512], F32, tag="ps",
                                         name=f"ps{j}")
                               for j in range(NJ)]
                        for ks in range(4):      # k slices of 8 x-tiles
                            xt_c = xt_pool.tile([128, 8, 512], BF16, tag="xt")
                            if b == 0 and tb == 0 and ks == 0:
                                # split the very first load: compute can
                                # start after just 2 x-tiles have landed
                                nc.sync.dma_start(
                                    xt_c[:, 0:2, :], xT[:, 0:2, c0:c0 + 512])
                                nc.sync.dma_start(
                                    xt_c[:, 2:8, :], xT[:, 2:8, c0:c0 + 512])
                            else:
                                nc.sync.dma_start(
                                    xt_c[:],
                                    xT[:, ks * 8:(ks + 1) * 8, c0:c0 + 512])
                            for k in range(8):
                                for j in range(NJ):
                                    nc.tensor.matmul(
                                        pss[j][:],
                                        wqkv[:, ks * 8 + k,
                                             j * HD:(j + 1) * HD],
                                        xt_c[:, k, :],
                                        start=(ks == 0 and k == 0),
                                        stop=(ks == 3 and k == 7))
                        # V first: DVE frees the V accumulator so the PE
                        # transposes can overlap the RoPE elementwise work
                        v_sb = vtmp_pool.tile([128, 512], BF16)
                        nc.vector.tensor_copy(v_sb[:], pss[NJ - 1][:])
                        tp4 = trps.tile([128, 4, 128], BF16)
                        for h2 in range(4):
                            nc.tensor.transpose(
                                tp4[:, h2, :],
                                v_sb[:, h2 * 128:(h2 + 1) * 128],
                                ident[:])
                        for j in range(HQ + 1):
                            ps = pss[j]
                            # RoPE: out = z*cos + swap64(z)*sin_signed
                            tmp = rope_pool.tile([128, 512], F32, tag="tmp")
                            nc.vector.tensor_mul(
                                tmp[0:64, :], ps[64:128, :], sin_c[0:64, :])
                            nc.vector.tensor_mul(
                                tmp[64:128, :], ps[0:64, :], sin_c[64:128, :])
                            t2 = rope_pool.tile([128, 512], F32, tag="t2")
                            nc.vector.tensor_mul(t2[:], ps[:], cos_c[:])
                            if j < HQ:
                                nc.vector.tensor_add(
                                    qT_s[:, j, sl], t2[:], tmp[:])
                            else:
                                nc.vector.tensor_add(
                                    kT_s[:, sl], t2[:], tmp[:])
                        nc.vector.tensor_copy(
                            v_s[:, tb * 4:tb * 4 + 4, :], tp4[:])

                # ---------- A: attention (writes att_h in SBUF) ----------
                # One flat software pipeline across all (qb, h, kt) steps:
                # the PE issues scores(i+1..i+DEPTH) while ACT exps pT(i),
                # so neither the exp latency nor the per-head pipeline
                # refill ever stalls the in-order PE. Diagonal block j
                # trims its q-range to [j*128, 512): everything earlier is
                # fully masked (exp == 0), so neither scores nor PV/sums
                # need to touch it.
                with (
                    tc.tile_pool(name="pT", bufs=6) as p_pool,
                    tc.tile_pool(name="rcp", bufs=2) as r_pool,
                    tc.tile_pool(name="sps", bufs=4, space="PSUM") as sps,
                    tc.tile_pool(name="sums", bufs=2, space="PSUM") as sums_ps,
                    tc.tile_pool(name="ops", bufs=2, space="PSUM") as o_ps_pool,
                ):
                    DEPTH = 3
                    acc = {}
                    pend = []

                    def a_flush(n):
                        while len(pend) > n:
                            pv, qb, h, kt, q0, last = pend.pop(0)
                            sum_ps, o_ps = acc[(qb, h)]
                            nc.tensor.matmul(
                                sum_ps[:, q0:], ones_col[:], pv[:, q0:],
                                start=(kt == 0), stop=last)
                            nc.tensor.matmul(
                                o_ps[:, q0:], v_s[:, kt, :], pv[:, q0:],
                                start=(kt == 0), stop=last)
                            if last:
                                recip = r_pool.tile([1, QW], F32, tag="rcp")
                                nc.vector.reciprocal(recip[:], sum_ps[:])
                                bc_sb = r_pool.tile([128, QW], F32, tag="bc")
                                nc.gpsimd.partition_broadcast(
                                    bc_sb[:], recip[:])
                                nc.vector.tensor_mul(
                                    att_h[:, h, qb * QW:(qb + 1) * QW],
                                    o_ps[:], bc_sb[:])
                                del acc[(qb, h)]

                    for qb in range(QB):
                        nkt = 4 * (qb + 1)
                        for h in range(HQ):
                            q0g = qb * QW          # global q offset
                            acc[(qb, h)] = (
                                sums_ps.tile([1, QW], F32, name="sum_ps"),
                                o_ps_pool.tile([128, QW], F32, name="o_ps"))
                            for kt in range(nkt):
                                j = kt - 4 * qb
                                q0 = j * 128 if j >= 0 else 0
                                s_ps = sps.tile([128, QW], F32, tag="s_ps")
                                nc.tensor.matmul(
                                    s_ps[:, q0:],
                                    kT_s[:, kt * 128:(kt + 1) * 128],
                                    qT_s[:, h, q0g + q0:q0g + QW],
                                    start=True, stop=True)
                                if j >= 0:
                                    nc.vector.tensor_add(
                                        s_ps[:, q0:q0 + 128],
                                        s_ps[:, q0:q0 + 128], tri[:])
                                pT = p_pool.tile([128, QW], BF16)
                                nc.scalar.activation(
                                    pT[:, q0:], s_ps[:, q0:], EXP, bias=0.0,
                                    scale=SCALE)
                                pend.append(
                                    (pT, qb, h, kt, q0, kt == nkt - 1))
                                a_flush(DEPTH)
                    a_flush(0)

                # ---------- W: output projection partial ----------
                with (
                    tc.tile_pool(name="osb", bufs=2) as osb_pool,
                    tc.tile_pool(name="wps", bufs=5, space="PSUM") as wps,
                ):
                    for mb in range(8):          # 512-wide output columns
                        for tg in range(4):      # groups of 4 token tiles
                            o_sb = osb_pool.tile([128, 4, 512], F32)
                            for ts in range(4):
                                tt = tg * 4 + ts
                                ps_w = wps.tile([128, 512], F32)
                                for d4 in range(HQ):
                                    nc.tensor.matmul(
                                        ps_w[:],
                                        att_h[:, d4, tt * 128:(tt + 1) * 128],
                                        wo_all[:, d4,
                                               mb * 512:(mb + 1) * 512],
                                        start=(d4 == 0), stop=(d4 == HQ - 1))
                                nc.vector.tensor_copy(o_sb[:, ts, :], ps_w[:])
                            g0 = b * (S // 128) + tg * 4
                            # stores go on the scalar queue so the sync
                            # queue can prefetch the next batch's x tiles
                            nc.scalar.dma_start(
                                out_v[:, g0:g0 + 4, mb * 512:(mb + 1) * 512],
                                o_sb[:])

    nc.compile()
    return nc


def _build_general():
    """Fallback for a non-causal mask: baseline f32r kernel, full mask."""
    nc = bacc.Bacc("TRN2", target_bir_lowering=False, debug=False)

    xT_d = nc.dram_tensor("xT", [DIM, TOK], F32R, kind="ExternalInput")
    w_d = nc.dram_tensor("wqkvT", [DIM, NJ * HD], F32R, kind="ExternalInput")
    wo_d = nc.dram_tensor("woT", [HQ * HD, DIM], F32R, kind="ExternalInput")
    cos_d = nc.dram_tensor("cosT", [HD, S], F32, kind="ExternalInput")
    sin_d = nc.dram_tensor("sinTs", [HD, S], F32, kind="ExternalInput")
    mask_d = nc.dram_tensor("maskTd", [QB, KT, 128, QW], F32,
                            kind="ExternalInput")
    out_d = nc.dram_tensor("out_part", [TOK, DIM], F32, kind="ExternalOutput")

    xT = xT_d.ap().rearrange("(kt p) t -> p kt t", p=128)
    w_ap = w_d.ap().rearrange("(kt p) j -> p kt j", p=128)
    wo_ap = wo_d.ap().rearrange("(dt p) m -> p dt m", p=128)
    out_v = out_d.ap().rearrange("(g p) m -> p g m", p=128)

    with tile.TileContext(nc) as tc:
        with (
            tc.tile_pool(name="const", bufs=1) as const_pool,
            tc.tile_pool(name="batch", bufs=1) as batch_pool,
            tc.tile_pool(name="dram", bufs=2, space="DRAM") as dram_pool,
        ):
            wqkv = const_pool.tile([128, 32, NJ * HD], F32R)
            for kc in range(4):
                nc.scalar.dma_start(wqkv[:, kc * 8:(kc + 1) * 8, :],
                                    w_ap[:, kc * 8:(kc + 1) * 8, :])
            ident = const_pool.tile([128, 128], F32)
            masks.make_identity(nc, ident[:])
            ones_f = const_pool.tile([128, 1], F32)
            nc.vector.memset(ones_f[:], 1.0)
            ones_col = const_pool.tile([128, 1], F32R)
            nc.vector.tensor_copy(ones_col[:], ones_f[:])

            kT_s = batch_pool.tile([128, S], F32R)
            v_s = batch_pool.tile([128, KT, HD], F32R)
            att_h = batch_pool.tile([128, HQ, S], F32R)

            for b in range(B):
                qT_d = dram_pool.tile([HQ, HD, S], F32R)

                with (
                    tc.tile_pool(name="xt", bufs=2) as xt_pool,
                    tc.tile_pool(name="cs", bufs=2) as cs_pool,
                    tc.tile_pool(name="rope", bufs=2) as rope_pool,
                    tc.tile_pool(name="vtmp", bufs=2) as vtmp_pool,
                    tc.tile_pool(name="p1ps", bufs=NJ, space="PSUM") as p1ps,
                    tc.tile_pool(name="trps", bufs=2, space="PSUM") as trps,
                ):
                    for tb in range(4):
                        c0 = b * S + tb * 512
                        sl = slice(tb * 512, tb * 512 + 512)
                        cos_c = cs_pool.tile([HD, 512], F32, tag="cos")
                        sin_c = cs_pool.tile([HD, 512], F32, tag="sin")
                        nc.sync.dma_start(cos_c[:], cos_d.ap()[:, sl])
                        nc.sync.dma_start(sin_c[:], sin_d.ap()[:, sl])
                        pss = [p1ps.tile([128, 512], F32, tag="ps",
                                         name=f"ps{j}")
                               for j in range(NJ)]
                        for ks in range(4):
                            xt_c = xt_pool.tile([128, 8, 512], F32R, tag="xt")
                            nc.sync.dma_start(
                                xt_c[:],
                                xT[:, ks * 8:(ks + 1) * 8, c0:c0 + 512])
                            for j in range(NJ):
                                for k in range(8):
                                    nc.tensor.matmul(
                                        pss[j][:],
                                        wqkv[:, ks * 8 + k,
                                             j * HD:(j + 1) * HD],
                                        xt_c[:, k, :],
                                        start=(ks == 0 and k == 0),
                                        stop=(ks == 3 and k == 7))
                        for j in range(NJ):
                            ps = pss[j]
                            if j < HQ + 1:
                                tmp = rope_pool.tile([128, 512], F32,
                                                     tag="tmp")
                                nc.vector.tensor_mul(
                                    tmp[0:64, :], ps[64:128, :],
                                    sin_c[0:64, :])
                                nc.vector.tensor_mul(
                                    tmp[64:128, :], ps[0:64, :],
                                    sin_c[64:128, :])
                                t2 = rope_pool.tile([128, 512], F32, tag="t2")
                                nc.vector.tensor_mul(t2[:], ps[:], cos_c[:])
                                if j < HQ:
                                    rT = rope_pool.tile([128, 512], F32R,
                                                        tag="rT")
                                    nc.vector.tensor_add(rT[:], t2[:], tmp[:])
                                    nc.sync.dma_start(qT_d[j, :, sl], rT[:])
                                else:
                                    nc.vector.tensor_add(
                                        kT_s[:, sl], t2[:], tmp[:])
                            else:
                                v_sb = vtmp_pool.tile([128, 512], F32)
                                nc.vector.tensor_copy(v_sb[:], ps[:])
                                for h2 in range(4):
                                    tp = trps.tile([128, 128], F32)
                                    nc.tensor.transpose(
                                        tp[:],
                                        v_sb[:, h2 * 128:(h2 + 1) * 128],
                                        ident[:])
                                    nc.vector.tensor_copy(
                                        v_s[:, tb * 4 + h2, :], tp[:])

                with (
                    tc.tile_pool(name="mask", bufs=1) as mask_pool,
                    tc.tile_pool(name="qh", bufs=3) as q_pool,
                    tc.tile_pool(name="pT", bufs=3) as p_pool,
                    tc.tile_pool(name="rcp", bufs=2) as r_pool,
                    tc.tile_pool(name="sps", bufs=3, space="PSUM") as sps,
                    tc.tile_pool(name="sums", bufs=2, space="PSUM") as sums_ps,
                    tc.tile_pool(name="ops", bufs=3, space="PSUM") as o_ps_pool,
                ):
                    for qb in range(QB):
                        m_s = mask_pool.tile([128, KT, QW], F32)
                        nc.scalar.dma_start(
                            m_s[:],
                            mask_d.ap()[qb].rearrange("kt p q -> p kt q"))
                        for h in range(HQ):
                            qh = q_pool.tile([128, QW], F32R)
                            nc.sync.dma_start(
                                qh[:], qT_d[h, :, qb * QW:(qb + 1) * QW])
                            sum_ps = sums_ps.tile([1, QW], F32)
                            o_ps = o_ps_pool.tile([128, QW], F32)
                            prev = None
                            for kt in range(KT):
                                s_ps = sps.tile([128, QW], F32, tag="s_ps")
                                nc.tensor.matmul(
                                    s_ps[:], kT_s[:, kt * 128:(kt + 1) * 128],
                                    qh[:], start=True, stop=True)
                                nc.vector.tensor_add(
                                    s_ps[:], s_ps[:], m_s[:, kt, :])
                                pT = p_pool.tile([128, QW], F32R)
                                nc.scalar.activation(
                                    pT[:], s_ps[:], EXP, bias=0.0,
                                    scale=SCALE)
                                if prev is not None:
                                    pv, pkt = prev
                                    nc.tensor.matmul(
                                        sum_ps[:], ones_col[:], pv[:],
                                        start=(pkt == 0), stop=False)
                                    nc.tensor.matmul(
                                        o_ps[:], v_s[:, pkt, :], pv[:],
                                        start=(pkt == 0), stop=False)
                                prev = (pT, kt)
                            pv, pkt = prev
                            nc.tensor.matmul(
                                sum_ps[:], ones_col[:], pv[:],
                                start=(pkt == 0), stop=True)
                            nc.tensor.matmul(
                                o_ps[:], v_s[:, pkt, :], pv[:],
                                start=(pkt == 0), stop=True)
                            recip = r_pool.tile([1, QW], F32, tag="rcp")
                            nc.vector.reciprocal(recip[:], sum_ps[:])
                            bc_sb = r_pool.tile([128, QW], F32, tag="bc")
                            nc.gpsimd.partition_broadcast(bc_sb[:], recip[:])
                            nc.vector.tensor_mul(
                                att_h[:, h, qb * QW:(qb + 1) * QW],
                                o_ps[:], bc_sb[:])

                with (
                    tc.tile_pool(name="wo", bufs=3) as wo_pool,
                    tc.tile_pool(name="osb", bufs=2) as osb_pool,
                    tc.tile_pool(name="wps", bufs=5, space="PSUM") as wps,
                ):
                    for mb in range(8):
                        wo_t = wo_pool.tile([128, HQ, 512], F32R)
                        nc.sync.dma_start(
                            wo_t[:], wo_ap[:, :, mb * 512:(mb + 1) * 512])
                        for tg in range(4):
                            o_sb = osb_pool.tile([128, 4, 512], F32)
                            for ts in range(4):
                                tt = tg * 4 + ts
                                ps_w = wps.tile([128, 512], F32)
                                for d4 in range(HQ):
                                    nc.tensor.matmul(
                                        ps_w[:],
                                        att_h[:, d4, tt * 128:(tt + 1) * 128],
                                        wo_t[:, d4, :],
                                        start=(d4 == 0), stop=(d4 == HQ - 1))
                                nc.vector.tensor_copy(o_sb[:, ts, :], ps_w[:])
                            g0 = b * (S // 128) + tg * 4
                            nc.sync.dma_start(
                                out_v[:, g0:g0 + 4, mb * 512:(mb + 1) * 512],
                                o_sb[:])

    nc.compile()
    return nc


_CACHE = {}
LAST_EXEC_NS = None


def _get_nc(causal: bool):
    if causal not in _CACHE:
        _CACHE[causal] = _build_causal() if causal else _build_general()
    return _CACHE[causal]


def _host_prep(x, wq, wk, wv, wo, freqs_cos, freqs_sin, mask):
    perm = np.concatenate([np.arange(0, HD, 2), np.arange(1, HD, 2)])
    wq_p = wq.reshape(NH, HD, DIM)[:, perm, :].reshape(NH * HD, DIM)
    wk_p = wk.reshape(NKV, HD, DIM)[:, perm, :].reshape(NKV * HD, DIM)

    xT = np.ascontiguousarray(x.reshape(TOK, DIM).T)

    cos = freqs_cos.T                     # [64, S]
    sin = freqs_sin.T
    cosT = np.ascontiguousarray(np.concatenate([cos, cos], 0))       # [128, S]
    sinTs = np.ascontiguousarray(np.concatenate([-sin, sin], 0))

    ref_mask = np.triu(np.full((S, S), -1e9, dtype=np.float32), k=1)
    causal = np.array_equal(mask, ref_mask)

    in_maps = []
    if causal:
        xTb = xT.astype(NPBF16)
        # diagonal-block triangle in [k, q] layout: -inf where k > q
        triM = np.tril(np.full((128, 128), -1e9 / np.float32(SCALE),
                               dtype=np.float32), -1)
        triM = np.ascontiguousarray(triM)
        for c in range(NCORES):
            wqT = wq_p[c * HQ * HD:(c + 1) * HQ * HD, :].T      # [DIM, 512]
            wkT = wk_p[c * HD:(c + 1) * HD, :].T                # [DIM, 128]
            wvT = wv[c * HD:(c + 1) * HD, :].T                  # [DIM, 128]
            wqkvT = np.ascontiguousarray(
                np.concatenate([wqT, wkT, wvT], 1)).astype(NPBF16)
            woT = np.ascontiguousarray(
                wo[:, c * HQ * HD:(c + 1) * HQ * HD].T).astype(NPBF16)
            in_maps.append({
                "xT": xTb, "wqkvT": wqkvT, "woT": woT,
                "cosT": cosT, "sinTs": sinTs, "triM": triM,
            })
        return causal, in_maps

    maskT = np.ascontiguousarray(mask.T) / np.float32(SCALE)   # [k, q]
    maskTd = np.empty((QB, KT, 128, QW), dtype=np.float32)
    for qb in range(QB):
        for j in range(KT):
            maskTd[qb, j] = maskT[j * 128:(j + 1) * 128,
                                  qb * QW:(qb + 1) * QW]
    for c in range(NCORES):
        wqT = wq_p[c * HQ * HD:(c + 1) * HQ * HD, :].T
        wkT = wk_p[c * HD:(c + 1) * HD, :].T
        wvT = wv[c * HD:(c + 1) * HD, :].T
        wqkvT = np.ascontiguousarray(np.concatenate([wqT, wkT, wvT], 1))
        woT = np.ascontiguousarray(wo[:, c * HQ * HD:(c + 1) * HQ * HD].T)
        in_maps.append({
            "xT": xT, "wqkvT": wqkvT, "woT": woT,
            "cosT": cosT, "sinTs": sinTs, "maskTd": maskTd,
        })
    return causal, in_maps


def kernel(x, wq, wk, wv, wo, freqs_cos, freqs_sin, mask, start_pos):
    global LAST_EXEC_NS
    causal, in_maps = _host_prep(
        np.asarray(x, np.float32), np.asarray(wq, np.float32),
        np.asarray(wk, np.float32), np.asarray(wv, np.float32),
        np.asarray(wo, np.float32), np.asarray(freqs_cos, np.float32),
        np.asarray(freqs_sin, np.float32), np.asarray(mask, np.float32))

    nc = _get_nc(causal)
    res = run_bass_kernel_spmd(nc, in_maps, core_ids=list(range(NCORES)))
    LAST_EXEC_NS = res.exec_time_ns

    acc = res.results[0]["out_part"].astype(np.float64)
    for c in range(1, NCORES):
        acc += res.results[c]["out_part"]
    return acc.astype(np.float32).reshape(B, S, DIM)


if __name__ == "__main__":
    rng = np.random.default_rng(0)
    inputs = {
        "x": rng.standard_normal((B, S, DIM), dtype=np.float32),
        "wq": (rng.standard_normal((DIM, DIM), dtype=np.float32) * 0.02),
        "wk": (rng.standard_normal((NKV * HD, DIM), dtype=np.float32) * 0.02),
        "wv": (rng.standard_normal((NKV * HD, DIM), dtype=np.float32) * 0.02),
        "wo": (rng.standard_normal((DIM, DIM), dtype=np.float32) * 0.02),
        "freqs_cos": rng.random((S, HD // 2), dtype=np.float32),
        "freqs_sin": rng.random((S, HD // 2), dtype=np.float32),
        "mask": np.triu(np.full((S, S), -1e9, dtype=np.float32), k=1),
        "start_pos": 0,
    }
    out = kernel(**inputs)
    print("out", out.shape, out.dtype, float(np.abs(out).mean()))


# revision 21
# speedup vs baseline: 1.0091x; 1.0091x over previous
"""Tensor-parallel llama-style attention (prefill) on 8 TRN2 NeuronCores.

Sharding: tensor-parallel over heads. Core c holds q-heads [4c, 4c+4),
kv-head c, the matching rows of wq/wk/wv, and columns [512c, 512c+512) of
wo. Each core computes a full-size partial of the output projection;
partials are summed on the host (the "all-reduce after wo").

Device-side layout (causal fast path):
  - All matmul operands are bf16 (PSUM accumulation stays fp32). bf16
    enables the compiler's fast-weight-load path and avoids the fp32
    PE power-throttle (HAM drops the PE clock to 1.2 GHz under
    sustained fp32-mode matmul); tolerance is 2e-2, bf16 lands ~1e-3.
  - Activations kept transposed (feature dim on partitions): xT
    [DIM, TOK], Q^T/K^T [128, S] per head, V in token-major chunks.
  - wo ([128, 4, DIM] bf16, 32 KiB/partition) and Q^T (16 KiB/partition)
    are SBUF-resident: no DRAM spill, no W-phase weight streaming.
  - RoPE: head-dim basis permuted on the host (even first, odd second),
    turning the interleaved rotation into a half-partition swap +
    elementwise mul/add against cos/sin tables, read from PSUM.
  - Causal mask: one [128,128] triangle tile (the diagonal-block
    pattern only depends on k-q). Off-diagonal blocks below the
    diagonal need no mask; blocks above are never computed. Diagonal
    block j additionally restricts its q-range to [j*128, 512), which
    recovers 128-granular causal savings (136/160 of the block-level
    work) while keeping 512-wide moving operands.
  - Softmax: no max-subtraction (scores*scale is O(10); exp safe in
    fp32). Row sums via a ones-vector matmul on the tensor engine; the
    reciprocal is partition-broadcast by gpsimd and applied by DVE.
"""

import math
import os
import sys

sys.path.insert(0, "/opt/trn_rl_repo")

import numpy as np
import ml_dtypes

import concourse.bacc as bacc
import concourse.tile as tile
import concourse.mybir as mybir
from concourse import masks
from concourse.bass_utils import run_bass_kernel_spmd

B, S, DIM = 2, 2048, 4096
TOK = B * S
NH, NKV, HD = 32, 8, 128
NCORES = 8
HQ = NH // NCORES            # 4 query heads per core
SCALE = 1.0 / math.sqrt(HD)
F32 = mybir.dt.float32
F32R = mybir.dt.float32r
BF16 = mybir.dt.bfloat16
NPBF16 = ml_dtypes.bfloat16
EXP = mybir.ActivationFunctionType.Exp

QB = 4          # q-blocks per batch (512 queries each)
QW = S // QB    # 512
KT = S // 128   # 16 k-tiles per batch
NJ = HQ + 2     # 6 projection output tiles: 4 Q heads, K, V


def _build_causal():
    nc = bacc.Bacc("TRN2", target_bir_lowering=False, debug=False)

    xT_d = nc.dram_tensor("xT", [DIM, TOK], BF16, kind="ExternalInput")
    w_d = nc.dram_tensor("wqkvT", [DIM, NJ * HD], BF16, kind="ExternalInput")
    wo_d = nc.dram_tensor("woT", [HQ * HD, DIM], BF16, kind="ExternalInput")
    cos_d = nc.dram_tensor("cosT", [HD, S], F32, kind="ExternalInput")
    sin_d = nc.dram_tensor("sinTs", [HD, S], F32, kind="ExternalInput")
    tri_d = nc.dram_tensor("triM", [128, 128], F32, kind="ExternalInput")
    out_d = nc.dram_tensor("out_part", [TOK, DIM], F32, kind="ExternalOutput")

    xT = xT_d.ap().rearrange("(kt p) t -> p kt t", p=128)      # [128, 32, TOK]
    w_ap = w_d.ap().rearrange("(kt p) j -> p kt j", p=128)     # [128, 32, 768]
    wo_ap = wo_d.ap().rearrange("(dt p) m -> p dt m", p=128)   # [128, 4, DIM]
    out_v = out_d.ap().rearrange("(g p) m -> p g m", p=128)    # [128, 32, DIM]

    with tile.TileContext(nc) as tc:
        with (
            tc.tile_pool(name="const", bufs=1) as const_pool,
            tc.tile_pool(name="batch", bufs=1) as batch_pool,
        ):
            wqkv = const_pool.tile([128, 32, NJ * HD], BF16)
            kc0 = 0
            for kcn in (2, 2, 4, 4, 4, 4, 4, 4, 4):  # small first chunks so
                nc.scalar.dma_start(wqkv[:, kc0:kc0 + kcn, :],  # P1 starts early
                                    w_ap[:, kc0:kc0 + kcn, :])
                kc0 += kcn
            wo_all = const_pool.tile([128, HQ, DIM], BF16)
            for mc in range(4):
                nc.gpsimd.dma_start(wo_all[:, :, mc * 1024:(mc + 1) * 1024],
                                    wo_ap[:, :, mc * 1024:(mc + 1) * 1024])
            tri = const_pool.tile([128, 128], F32)
            nc.gpsimd.dma_start(tri[:], tri_d.ap())
            ident = const_pool.tile([128, 128], BF16)
            masks.make_identity(nc, ident[:])
            ones_col = const_pool.tile([128, 1], BF16)
            nc.vector.memset(ones_col[:], 1.0)

            # per-batch SBUF-resident tensors, split per 512-token chunk:
            # dependency tracking is per-tile, so a single [128, S] tensor
            # would stall attention qb=0 on the *last* chunk's RoPE
            kT_t = [batch_pool.tile([128, 512], BF16, name=f"kT{t}")
                    for t in range(4)]
            v_t = [batch_pool.tile([128, 4, HD], BF16, name=f"v{t}")
                   for t in range(4)]
            qT_t = [batch_pool.tile([128, HQ, 512], BF16, name=f"qT{t}")
                    for t in range(4)]
            att_q = [batch_pool.tile([128, HQ, 512], BF16, name=f"att{t}")
                     for t in range(4)]

            for b in range(B):
                # ---------- P1: QKV projections + RoPE ----------
                with (
                    tc.tile_pool(name="xt", bufs=2) as xt_pool,
                    tc.tile_pool(name="cs", bufs=2) as cs_pool,
                    tc.tile_pool(name="rope", bufs=2) as rope_pool,
                    tc.tile_pool(name="vtmp", bufs=2) as vtmp_pool,
                    tc.tile_pool(name="p1ps", bufs=NJ, space="PSUM") as p1ps,
                    tc.tile_pool(name="trps", bufs=2, space="PSUM") as trps,
                ):
                    for tb in range(4):          # 512-token chunks
                        c0 = b * S + tb * 512
                        sl = slice(tb * 512, tb * 512 + 512)
                        cos_c = cs_pool.tile([HD, 512], F32, tag="cos")
                        sin_c = cs_pool.tile([HD, 512], F32, tag="sin")
                        nc.gpsimd.dma_start(cos_c[:], cos_d.ap()[:, sl])
                        nc.gpsimd.dma_start(sin_c[:], sin_d.ap()[:, sl])
                        pss = [p1ps.tile([128, 512], F32, tag="ps",
                                         name=f"ps{j}")
                               for j in range(NJ)]
                        for ks in range(4):      # k slices of 8 x-tiles
                            xt_c = xt_pool.tile([128, 8, 512], BF16, tag="xt")
                            if b == 0 and tb == 0 and ks == 0:
                                # split the very first load: compute can
                                # start after just 2 x-tiles have landed
                                nc.sync.dma_start(
                                    xt_c[:, 0:2, :], xT[:, 0:2, c0:c0 + 512])
                                nc.sync.dma_start(
                                    xt_c[:, 2:8, :], xT[:, 2:8, c0:c0 + 512])
                            else:
                                nc.sync.dma_start(
                                    xt_c[:],
                                    xT[:, ks * 8:(ks + 1) * 8, c0:c0 + 512])
                            for k in range(8):
                                for j in range(NJ):
                                    nc.tensor.matmul(
                                        pss[j][:],
                                        wqkv[:, ks * 8 + k,
                                             j * HD:(j + 1) * HD],
                                        xt_c[:, k, :],
                                        start=(ks == 0 and k == 0),
                                        stop=(ks == 3 and k == 7))
                        # V first: DVE frees the V accumulator so the PE
                        # transposes can overlap the RoPE elementwise work
                        v_sb = vtmp_pool.tile([128, 512], BF16)
                        nc.vector.tensor_copy(v_sb[:], pss[NJ - 1][:])
                        tp4 = trps.tile([128, 4, 128], BF16)
                        for h2 in range(4):
                            nc.tensor.transpose(
                                tp4[:, h2, :],
                                v_sb[:, h2 * 128:(h2 + 1) * 128],
                                ident[:])
                        for j in range(HQ + 1):
                            ps = pss[j]
                            # RoPE: out = z*cos + swap64(z)*sin_signed
                            tmp = rope_pool.tile([128, 512], F32, tag="tmp")
                            nc.vector.tensor_mul(
                                tmp[0:64, :], ps[64:128, :], sin_c[0:64, :])
                            nc.vector.tensor_mul(
                                tmp[64:128, :], ps[0:64, :], sin_c[64:128, :])
                            t2 = rope_pool.tile([128, 512], F32, tag="t2")
                            nc.vector.tensor_mul(t2[:], ps[:], cos_c[:])
                            if j < HQ:
                                nc.vector.tensor_add(
                                    qT_t[tb][:, j, :], t2[:], tmp[:])
                            else:
                                nc.vector.tensor_add(
                                    kT_t[tb][:], t2[:], tmp[:])
                        nc.vector.tensor_copy(v_t[tb][:], tp4[:])

                # ---------- A: attention (writes att_h in SBUF) ----------
                # One flat software pipeline across all (qb, h, kt) steps:
                # the PE issues scores(i+1..i+DEPTH) while ACT exps pT(i),
                # so neither the exp latency nor the per-head pipeline
                # refill ever stalls the in-order PE. Diagonal block j
                # trims its q-range to [j*128, 512): everything earlier is
                # fully masked (exp == 0), so neither scores nor PV/sums
                # need to touch it.
                with (
                    tc.tile_pool(name="pT", bufs=7) as p_pool,
                    tc.tile_pool(name="rcp", bufs=2) as r_pool,
                    tc.tile_pool(name="sps", bufs=4, space="PSUM") as sps,
                    tc.tile_pool(name="sums", bufs=2, space="PSUM") as sums_ps,
                    tc.tile_pool(name="ops", bufs=2, space="PSUM") as o_ps_pool,
                ):
                    DEPTH = 4
                    acc = {}
                    pend = []

                    def a_flush(n):
                        while len(pend) > n:
                            pv, qb, h, kt, q0, last = pend.pop(0)
                            sum_ps, o_ps = acc[(qb, h)]
                            nc.tensor.matmul(
                                sum_ps[:, q0:], ones_col[:], pv[:, q0:],
                                start=(kt == 0), stop=last)
                            nc.tensor.matmul(
                                o_ps[:, q0:], v_t[kt // 4][:, kt % 4, :],
                                pv[:, q0:],
                                start=(kt == 0), stop=last)
                            if last:
                                recip = r_pool.tile([1, QW], F32, tag="rcp")
                                nc.vector.reciprocal(recip[:], sum_ps[:])
                                bc_sb = r_pool.tile([128, QW], F32, tag="bc")
                                nc.gpsimd.partition_broadcast(
                                    bc_sb[:], recip[:])
                                nc.vector.tensor_mul(
                                    att_q[qb][:, h, :], o_ps[:], bc_sb[:])
                                del acc[(qb, h)]

                    for qb in range(QB):
                        nkt = 4 * (qb + 1)
                        for h in range(HQ):
                            acc[(qb, h)] = (
                                sums_ps.tile([1, QW], F32, name="sum_ps"),
                                o_ps_pool.tile([128, QW], F32, name="o_ps"))
                            for kt in range(nkt):
                                j = kt - 4 * qb
                                q0 = j * 128 if j >= 0 else 0
                                ki = kt % 4
                                s_ps = sps.tile([128, QW], F32, tag="s_ps")
                                nc.tensor.matmul(
                                    s_ps[:, q0:],
                                    kT_t[kt // 4][:, ki * 128:(ki + 1) * 128],
                                    qT_t[qb][:, h, q0:],
                                    start=True, stop=True)
                                if j >= 0:
                                    nc.vector.tensor_add(
                                        s_ps[:, q0:q0 + 128],
                                        s_ps[:, q0:q0 + 128], tri[:])
                                pT = p_pool.tile([128, QW], BF16)
                                nc.scalar.activation(
                                    pT[:, q0:], s_ps[:, q0:], EXP, bias=0.0,
                                    scale=SCALE)
                                pend.append(
                                    (pT, qb, h, kt, q0, kt == nkt - 1))
                                a_flush(DEPTH)
                    a_flush(0)

                # ---------- W: output projection partial ----------
                with (
                    tc.tile_pool(name="osb", bufs=2) as osb_pool,
                    tc.tile_pool(name="wps", bufs=5, space="PSUM") as wps,
                ):
                    for mb in range(8):          # 512-wide output columns
                        m0 = mb * 512
                        for tg in range(4):      # groups of 4 token tiles
                            o_sb = osb_pool.tile([128, 4, 512], F32)
                            for ts in range(4):
                                tt = tg * 4 + ts
                                ti = tt % 4
                                ps_w = wps.tile([128, 512], F32)
                                for d4 in range(HQ):
                                    nc.tensor.matmul(
                                        ps_w[:],
                                        att_q[tt // 4][
                                            :, d4, ti * 128:(ti + 1) * 128],
                                        wo_all[:, d4, m0:m0 + 512],
                                        start=(d4 == 0), stop=(d4 == HQ - 1))
                                nc.vector.tensor_copy(o_sb[:, ts, :], ps_w[:])
                            g0 = b * (S // 128) + tg * 4
                            # stores go on the scalar queue so the sync
                            # queue can prefetch the next batch's x tiles
                            nc.scalar.dma_start(
                                out_v[:, g0:g0 + 4, m0:m0 + 512],
                                o_sb[:])

    nc.compile()
    return nc


def _build_general():
    """Fallback for a non-causal mask: baseline f32r kernel, full mask."""
    nc = bacc.Bacc("TRN2", target_bir_lowering=False, debug=False)

    xT_d = nc.dram_tensor("xT", [DIM, TOK], F32R, kind="ExternalInput")
    w_d = nc.dram_tensor("wqkvT", [DIM, NJ * HD], F32R, kind="ExternalInput")
    wo_d = nc.dram_tensor("woT", [HQ * HD, DIM], F32R, kind="ExternalInput")
    cos_d = nc.dram_tensor("cosT", [HD, S], F32, kind="ExternalInput")
    sin_d = nc.dram_tensor("sinTs", [HD, S], F32, kind="ExternalInput")
    mask_d = nc.dram_tensor("maskTd", [QB, KT, 128, QW], F32,
                            kind="ExternalInput")
    out_d = nc.dram_tensor("out_part", [TOK, DIM], F32, kind="ExternalOutput")

    xT = xT_d.ap().rearrange("(kt p) t -> p kt t", p=128)
    w_ap = w_d.ap().rearrange("(kt p) j -> p kt j", p=128)
    wo_ap = wo_d.ap().rearrange("(dt p) m -> p dt m", p=128)
    out_v = out_d.ap().rearrange("(g p) m -> p g m", p=128)

    with tile.TileContext(nc) as tc:
        with (
            tc.tile_pool(name="const", bufs=1) as const_pool,
            tc.tile_pool(name="batch", bufs=1) as batch_pool,
            tc.tile_pool(name="dram", bufs=2, space="DRAM") as dram_pool,
        ):
            wqkv = const_pool.tile([128, 32, NJ * HD], F32R)
            for kc in range(4):
                nc.scalar.dma_start(wqkv[:, kc * 8:(kc + 1) * 8, :],
                                    w_ap[:, kc * 8:(kc + 1) * 8, :])
            ident = const_pool.tile([128, 128], F32)
            masks.make_identity(nc, ident[:])
            ones_f = const_pool.tile([128, 1], F32)
            nc.vector.memset(ones_f[:], 1.0)
            ones_col = const_pool.tile([128, 1], F32R)
            nc.vector.tensor_copy(ones_col[:], ones_f[:])

            kT_s = batch_pool.tile([128, S], F32R)
            v_s = batch_pool.tile([128, KT, HD], F32R)
            att_h = batch_pool.tile([128, HQ, S], F32R)

            for b in range(B):
                qT_d = dram_pool.tile([HQ, HD, S], F32R)

                with (
                    tc.tile_pool(name="xt", bufs=2) as xt_pool,
                    tc.tile_pool(name="cs", bufs=2) as cs_pool,
                    tc.tile_pool(name="rope", bufs=2) as rope_pool,
                    tc.tile_pool(name="vtmp", bufs=2) as vtmp_pool,
                    tc.tile_pool(name="p1ps", bufs=NJ, space="PSUM") as p1ps,
                    tc.tile_pool(name="trps", bufs=2, space="PSUM") as trps,
                ):
                    for tb in range(4):
                        c0 = b * S + tb * 512
                        sl = slice(tb * 512, tb * 512 + 512)
                        cos_c = cs_pool.tile([HD, 512], F32, tag="cos")
                        sin_c = cs_pool.tile([HD, 512], F32, tag="sin")
                        nc.sync.dma_start(cos_c[:], cos_d.ap()[:, sl])
                        nc.sync.dma_start(sin_c[:], sin_d.ap()[:, sl])
                        pss = [p1ps.tile([128, 512], F32, tag="ps",
                                         name=f"ps{j}")
                               for j in range(NJ)]
                        for ks in range(4):
                            xt_c = xt_pool.tile([128, 8, 512], F32R, tag="xt")
                            nc.sync.dma_start(
                                xt_c[:],
                                xT[:, ks * 8:(ks + 1) * 8, c0:c0 + 512])
                            for j in range(NJ):
                                for k in range(8):
                                    nc.tensor.matmul(
                                        pss[j][:],
                                        wqkv[:, ks * 8 + k,
                                             j * HD:(j + 1) * HD],
                                        xt_c[:, k, :],
                                        start=(ks == 0 and k == 0),
                                        stop=(ks == 3 and k == 7))
                        for j in range(NJ):
                            ps = pss[j]
                            if j < HQ + 1:
                                tmp = rope_pool.tile([128, 512], F32,
                                                     tag="tmp")
                                nc.vector.tensor_mul(
                                    tmp[0:64, :], ps[64:128, :],
                                    sin_c[0:64, :])
                                nc.vector.tensor_mul(
                                    tmp[64:128, :], ps[0:64, :],
                                    sin_c[64:128, :])
                                t2 = rope_pool.tile([128, 512], F32, tag="t2")
                                nc.vector.tensor_mul(t2[:], ps[:], cos_c[:])
                                if j < HQ:
                                    rT = rope_pool.tile([128, 512], F32R,
                                                        tag="rT")
                                    nc.vector.tensor_add(rT[:], t2[:], tmp[:])
                                    nc.sync.dma_start(qT_d[j, :, sl], rT[:])
                                else:
                                    nc.vector.tensor_add(
                                        kT_s[:, sl], t2[:], tmp[:])
                            else:
                                v_sb = vtmp_pool.tile([128, 512], F32)
                                nc.vector.tensor_copy(v_sb[:], ps[:])
                                for h2 in range(4):
                                    tp = trps.tile([128, 128], F32)
                                    nc.tensor.transpose(
                                        tp[:],
                                        v_sb[:, h2 * 128:(h2 + 1) * 128],
                                        ident[:])
                                    nc.vector.tensor_copy(
                                        v_s[:, tb * 4 + h2, :], tp[:])

                with (
                    tc.tile_pool(name="mask", bufs=1) as mask_pool,
                    tc.tile_pool(name="qh", bufs=3) as q_pool,
                    tc.tile_pool(name="pT", bufs=3) as p_pool,
                    tc.tile_pool(name="rcp", bufs=2) as r_pool,
                    tc.tile_pool(name="sps", bufs=3, space="PSUM") as sps,
                    tc.tile_pool(name="sums", bufs=2, space="PSUM") as sums_ps,
                    tc.tile_pool(name="ops", bufs=3, space="PSUM") as o_ps_pool,
                ):
                    for qb in range(QB):
                        m_s = mask_pool.tile([128, KT, QW], F32)
                        nc.scalar.dma_start(
                            m_s[:],
                            mask_d.ap()[qb].rearrange("kt p q -> p kt q"))
                        for h in range(HQ):
                            qh = q_pool.tile([128, QW], F32R)
                            nc.sync.dma_start(
                                qh[:], qT_d[h, :, qb * QW:(qb + 1) * QW])
                            sum_ps = sums_ps.tile([1, QW], F32)
                            o_ps = o_ps_pool.tile([128, QW], F32)
                            prev = None
                            for kt in range(KT):
                                s_ps = sps.tile([128, QW], F32, tag="s_ps")
                                nc.tensor.matmul(
                                    s_ps[:], kT_s[:, kt * 128:(kt + 1) * 128],
                                    qh[:], start=True, stop=True)
                                nc.vector.tensor_add(
                                    s_ps[:], s_ps[:], m_s[:, kt, :])
                                pT = p_pool.tile([128, QW], F32R)
                                nc.scalar.activation(
                                    pT[:], s_ps[:], EXP, bias=0.0,
                                    scale=SCALE)
                                if prev is not None:
                                    pv, pkt = prev
                                    nc.tensor.matmul(
                                        sum_ps[:], ones_col[:], pv[:],
                                        start=(pkt == 0), stop=False)
                                    nc.tensor.matmul(
                                        o_ps[:], v_s[:, pkt, :], pv[:],
                                        start=(pkt == 0), stop=False)
                                prev = (pT, kt)
                            pv, pkt = prev
                            nc.tensor.matmul(
                                sum_ps[:], ones_col[:], pv[:],
                                start=(pkt == 0), stop=True)
                            nc.tensor.matmul(
                                o_ps[:], v_s[:, pkt, :], pv[:],
                                start=(pkt == 0), stop=True)
                            recip = r_pool.tile([1, QW], F32, tag="rcp")
                            nc.vector.reciprocal(recip[:], sum_ps[:])
                            bc_sb = r_pool.tile([128, QW], F32, tag="bc")
                            nc.gpsimd.partition_broadcast(bc_sb[:], recip[:])
                            nc.vector.tensor_mul(
                                att_h[:, h, qb * QW:(qb + 1) * QW],
                                o_ps[:], bc_sb[:])

                with (
                    tc.tile_pool(name="wo", bufs=3) as wo_pool,
                    tc.tile_pool(name="osb", bufs=2) as osb_pool,
                    tc.tile_pool(name="wps", bufs=5, space="PSUM") as wps,
                ):
                    for mb in range(8):
                        wo_t = wo_pool.tile([128, HQ, 512], F32R)
                        nc.sync.dma_start(
                            wo_t[:], wo_ap[:, :, mb * 512:(mb + 1) * 512])
                        for tg in range(4):
                            o_sb = osb_pool.tile([128, 4, 512], F32)
                            for ts in range(4):
                                tt = tg * 4 + ts
                                ps_w = wps.tile([128, 512], F32)
                                for d4 in range(HQ):
                                    nc.tensor.matmul(
                                        ps_w[:],
                                        att_h[:, d4, tt * 128:(tt + 1) * 128],
                                        wo_t[:, d4, :],
                                        start=(d4 == 0), stop=(d4 == HQ - 1))
                                nc.vector.tensor_copy(o_sb[:, ts, :], ps_w[:])
                            g0 = b * (S // 128) + tg * 4
                            nc.sync.dma_start(
                                out_v[:, g0:g0 + 4, mb * 512:(mb + 1) * 512],
                                o_sb[:])

    nc.compile()
    return nc


_CACHE = {}
LAST_EXEC_NS = None


def _get_nc(causal: bool):
    if causal not in _CACHE:
        _CACHE[causal] = _build_causal() if causal else _build_general()
    return _CACHE[causal]


def _host_prep(x, wq, wk, wv, wo, freqs_cos, freqs_sin, mask):
    perm = np.concatenate([np.arange(0, HD, 2), np.arange(1, HD, 2)])
    wq_p = wq.reshape(NH, HD, DIM)[:, perm, :].reshape(NH * HD, DIM)
    wk_p = wk.reshape(NKV, HD, DIM)[:, perm, :].reshape(NKV * HD, DIM)

    xT = np.ascontiguousarray(x.reshape(TOK, DIM).T)

    cos = freqs_cos.T                     # [64, S]
    sin = freqs_sin.T
    cosT = np.ascontiguousarray(np.concatenate([cos, cos], 0))       # [128, S]
    sinTs = np.ascontiguousarray(np.concatenate([-sin, sin], 0))

    ref_mask = np.triu(np.full((S, S), -1e9, dtype=np.float32), k=1)
    causal = np.array_equal(mask, ref_mask)

    in_maps = []
    if causal:
        xTb = xT.astype(NPBF16)
        # diagonal-block triangle in [k, q] layout: -inf where k > q
        triM = np.tril(np.full((128, 128), -1e9 / np.float32(SCALE),
                               dtype=np.float32), -1)
        triM = np.ascontiguousarray(triM)
        for c in range(NCORES):
            wqT = wq_p[c * HQ * HD:(c + 1) * HQ * HD, :].T      # [DIM, 512]
            wkT = wk_p[c * HD:(c + 1) * HD, :].T                # [DIM, 128]
            wvT = wv[c * HD:(c + 1) * HD, :].T                  # [DIM, 128]
            wqkvT = np.ascontiguousarray(
                np.concatenate([wqT, wkT, wvT], 1)).astype(NPBF16)
            woT = np.ascontiguousarray(
                wo[:, c * HQ * HD:(c + 1) * HQ * HD].T).astype(NPBF16)
            in_maps.append({
                "xT": xTb, "wqkvT": wqkvT, "woT": woT,
                "cosT": cosT, "sinTs": sinTs, "triM": triM,
            })
        return causal, in_maps

    maskT = np.ascontiguousarray(mask.T) / np.float32(SCALE)   # [k, q]
    maskTd = np.empty((QB, KT, 128, QW), dtype=np.float32)
    for qb in range(QB):
        for j in range(KT):
            maskTd[qb, j] = maskT[j * 128:(j + 1) * 128,
                                  qb * QW:(qb + 1) * QW]
    for c in range(NCORES):
        wqT = wq_p[c * HQ * HD:(c + 1) * HQ * HD, :].T
        wkT = wk_p[c * HD:(c + 1) * HD, :].T
        wvT = wv[c * HD:(c + 1) * HD, :].T
        wqkvT = np.ascontiguousarray(np.concatenate([wqT, wkT, wvT], 1))
        woT = np.ascontiguousarray(wo[:, c * HQ * HD:(c + 1) * HQ * HD].T)
        in_maps.append({
            "xT": xT, "wqkvT": wqkvT, "woT": woT,
            "cosT": cosT, "sinTs": sinTs, "maskTd": maskTd,
        })
    return causal, in_maps


def kernel(x, wq, wk, wv, wo, freqs_cos, freqs_sin, mask, start_pos):
    global LAST_EXEC_NS
    causal, in_maps = _host_prep(
        np.asarray(x, np.float32), np.asarray(wq, np.float32),
        np.asarray(wk, np.float32), np.asarray(wv, np.float32),
        np.asarray(wo, np.float32), np.asarray(freqs_cos, np.float32),
        np.asarray(freqs_sin, np.float32), np.asarray(mask, np.float32))

    nc = _get_nc(causal)
    res = run_bass_kernel_spmd(nc, in_maps, core_ids=list(range(NCORES)))
    LAST_EXEC_NS = res.exec_time_ns

    acc = res.results[0]["out_part"].astype(np.float64)
    for c in range(1, NCORES):
        acc += res.results[c]["out_part"]
    return acc.astype(np.float32).reshape(B, S, DIM)


if __name__ == "__main__":
    rng = np.random.default_rng(0)
    inputs = {
        "x": rng.standard_normal((B, S, DIM), dtype=np.float32),
        "wq": (rng.standard_normal((DIM, DIM), dtype=np.float32) * 0.02),
        "wk": (rng.standard_normal((NKV * HD, DIM), dtype=np.float32) * 0.02),
        "wv": (rng.standard_normal((NKV * HD, DIM), dtype=np.float32) * 0.02),
        "wo": (rng.standard_normal((DIM, DIM), dtype=np.float32) * 0.02),
        "freqs_cos": rng.random((S, HD // 2), dtype=np.float32),
        "freqs_sin": rng.random((S, HD // 2), dtype=np.float32),
        "mask": np.triu(np.full((S, S), -1e9, dtype=np.float32), k=1),
        "start_pos": 0,
    }
    out = kernel(**inputs)
    print("out", out.shape, out.dtype, float(np.abs(out).mean()))


# revision 24
# speedup vs baseline: 1.0437x; 1.0342x over previous
"""Tensor-parallel llama-style attention (prefill) on 8 TRN2 NeuronCores.

Sharding: tensor-parallel over heads. Core c holds q-heads [4c, 4c+4),
kv-head c, the matching rows of wq/wk/wv, and columns [512c, 512c+512) of
wo. Each core computes a full-size partial of the output projection;
partials are summed on the host (the "all-reduce after wo").

Device-side layout (causal fast path):
  - All matmul operands are bf16 (PSUM accumulation stays fp32). bf16
    enables the compiler's fast-weight-load path and avoids the fp32
    PE power-throttle (HAM drops the PE clock to 1.2 GHz under
    sustained fp32-mode matmul); tolerance is 2e-2, bf16 lands ~1e-3.
  - Activations kept transposed (feature dim on partitions): xT
    [DIM, TOK], Q^T/K^T [128, S] per head, V in token-major chunks.
  - wo ([128, 4, DIM] bf16, 32 KiB/partition) and Q^T (16 KiB/partition)
    are SBUF-resident: no DRAM spill, no W-phase weight streaming.
  - RoPE: head-dim basis permuted on the host (even first, odd second),
    turning the interleaved rotation into a half-partition swap +
    elementwise mul/add against cos/sin tables, read from PSUM.
  - Causal mask: one [128,128] triangle tile (the diagonal-block
    pattern only depends on k-q). Off-diagonal blocks below the
    diagonal need no mask; blocks above are never computed. Diagonal
    block j additionally restricts its q-range to [j*128, 512), which
    recovers 128-granular causal savings (136/160 of the block-level
    work) while keeping 512-wide moving operands.
  - Softmax: no max-subtraction (scores*scale is O(10); exp safe in
    fp32). Row sums via a ones-vector matmul on the tensor engine; the
    reciprocal is partition-broadcast by gpsimd and applied by DVE.
"""

import math
import os
import sys

sys.path.insert(0, "/opt/trn_rl_repo")

import numpy as np
import ml_dtypes

import concourse.bacc as bacc
import concourse.tile as tile
import concourse.mybir as mybir
from concourse import masks
from concourse.bass_utils import run_bass_kernel_spmd

B, S, DIM = 2, 2048, 4096
TOK = B * S
NH, NKV, HD = 32, 8, 128
NCORES = 8
HQ = NH // NCORES            # 4 query heads per core
SCALE = 1.0 / math.sqrt(HD)
F32 = mybir.dt.float32
F32R = mybir.dt.float32r
BF16 = mybir.dt.bfloat16
NPBF16 = ml_dtypes.bfloat16
EXP = mybir.ActivationFunctionType.Exp

QB = 4          # q-blocks per batch (512 queries each)
QW = S // QB    # 512
KT = S // 128   # 16 k-tiles per batch
NJ = HQ + 2     # 6 projection output tiles: 4 Q heads, K, V


def _build_causal():
    nc = bacc.Bacc("TRN2", target_bir_lowering=False, debug=False)

    xT_d = nc.dram_tensor("xT", [DIM, TOK], BF16, kind="ExternalInput")
    w_d = nc.dram_tensor("wqkvT", [DIM, NJ * HD], BF16, kind="ExternalInput")
    wo_d = nc.dram_tensor("woT", [HQ * HD, DIM], BF16, kind="ExternalInput")
    cos_d = nc.dram_tensor("cosT", [HD, S], F32, kind="ExternalInput")
    sin_d = nc.dram_tensor("sinTs", [HD, S], F32, kind="ExternalInput")
    tri_d = nc.dram_tensor("triM", [128, 128], F32, kind="ExternalInput")
    out_d = nc.dram_tensor("out_part", [TOK, DIM], F32, kind="ExternalOutput")

    xT = xT_d.ap().rearrange("(kt p) t -> p kt t", p=128)      # [128, 32, TOK]
    w_ap = w_d.ap().rearrange("(kt p) j -> p kt j", p=128)     # [128, 32, 768]
    wo_ap = wo_d.ap().rearrange("(dt p) m -> p dt m", p=128)   # [128, 4, DIM]
    out_v = out_d.ap().rearrange("(g p) m -> p g m", p=128)    # [128, 32, DIM]

    with tile.TileContext(nc) as tc:
        with (
            tc.tile_pool(name="const", bufs=1) as const_pool,
            tc.tile_pool(name="batch", bufs=1) as batch_pool,
        ):
            wqkv = const_pool.tile([128, 32, NJ * HD], BF16)
            kc0 = 0
            for kcn in (2, 2, 4, 4, 4, 4, 4, 4, 4):  # small first chunks so
                nc.scalar.dma_start(wqkv[:, kc0:kc0 + kcn, :],  # P1 starts early
                                    w_ap[:, kc0:kc0 + kcn, :])
                kc0 += kcn
            wo_all = const_pool.tile([128, HQ, DIM], BF16)
            for mc in range(4):
                nc.gpsimd.dma_start(wo_all[:, :, mc * 1024:(mc + 1) * 1024],
                                    wo_ap[:, :, mc * 1024:(mc + 1) * 1024])
            tri = const_pool.tile([128, 128], F32)
            nc.gpsimd.dma_start(tri[:], tri_d.ap())
            ident = const_pool.tile([128, 128], BF16)
            masks.make_identity(nc, ident[:])
            ones_col = const_pool.tile([128, 1], BF16)
            nc.vector.memset(ones_col[:], 1.0)

            # per-batch SBUF-resident tensors, split per 512-token chunk:
            # dependency tracking is per-tile, so a single [128, S] tensor
            # would stall attention qb=0 on the *last* chunk's RoPE
            kT_t = [batch_pool.tile([128, 512], BF16, name=f"kT{t}")
                    for t in range(4)]
            v_t = [batch_pool.tile([128, 4, HD], BF16, name=f"v{t}")
                   for t in range(4)]
            qT_t = [batch_pool.tile([128, HQ, 512], BF16, name=f"qT{t}")
                    for t in range(4)]
            att_q = [batch_pool.tile([128, HQ, 512], BF16, name=f"att{t}")
                     for t in range(4)]

            for b in range(B):
                # ---------- P1: QKV projections + RoPE ----------
                with (
                    tc.tile_pool(name="xt", bufs=3) as xt_pool,
                    tc.tile_pool(name="cs", bufs=2) as cs_pool,
                    tc.tile_pool(name="rope", bufs=2) as rope_pool,
                    tc.tile_pool(name="vtmp", bufs=2) as vtmp_pool,
                    tc.tile_pool(name="p1ps", bufs=NJ, space="PSUM") as p1ps,
                    tc.tile_pool(name="trps", bufs=2, space="PSUM") as trps,
                ):
                    for tb in range(4):          # 512-token chunks
                        c0 = b * S + tb * 512
                        sl = slice(tb * 512, tb * 512 + 512)
                        cos_c = cs_pool.tile([HD, 512], F32, tag="cos")
                        sin_c = cs_pool.tile([HD, 512], F32, tag="sin")
                        nc.gpsimd.dma_start(cos_c[:], cos_d.ap()[:, sl])
                        nc.gpsimd.dma_start(sin_c[:], sin_d.ap()[:, sl])
                        pss = [p1ps.tile([128, 512], F32, tag="ps",
                                         name=f"ps{j}")
                               for j in range(NJ)]

                        def rope(j, ps):
                            # RoPE: out = z*cos + swap64(z)*sin_signed
                            tmp = rope_pool.tile([128, 512], F32, tag="tmp")
                            nc.vector.tensor_mul(
                                tmp[0:64, :], ps[64:128, :], sin_c[0:64, :])
                            nc.vector.tensor_mul(
                                tmp[64:128, :], ps[0:64, :], sin_c[64:128, :])
                            t2 = rope_pool.tile([128, 512], F32, tag="t2")
                            nc.vector.tensor_mul(t2[:], ps[:], cos_c[:])
                            if j < HQ:
                                nc.vector.tensor_add(
                                    qT_t[tb][:, j, :], t2[:], tmp[:])
                            else:
                                nc.vector.tensor_add(
                                    kT_t[tb][:], t2[:], tmp[:])

                        # pass 1 (k slices 0..15): k-outer, all 6 outputs
                        for ks in range(2):
                            xt_c = xt_pool.tile([128, 8, 512], BF16, tag="xt")
                            if b == 0 and tb == 0 and ks == 0:
                                # split the very first load: compute can
                                # start after just 2 x-tiles have landed
                                nc.sync.dma_start(
                                    xt_c[:, 0:2, :], xT[:, 0:2, c0:c0 + 512])
                                nc.sync.dma_start(
                                    xt_c[:, 2:8, :], xT[:, 2:8, c0:c0 + 512])
                            else:
                                nc.sync.dma_start(
                                    xt_c[:],
                                    xT[:, ks * 8:(ks + 1) * 8, c0:c0 + 512])
                            for k in range(8):
                                for j in range(NJ):
                                    nc.tensor.matmul(
                                        pss[j][:],
                                        wqkv[:, ks * 8 + k,
                                             j * HD:(j + 1) * HD],
                                        xt_c[:, k, :],
                                        start=(ks == 0 and k == 0),
                                        stop=False)
                        # pass 2 (k slices 16..31): j-outer so each output
                        # finishes (and its RoPE drains its PSUM bank)
                        # while the remaining outputs are still matmuling.
                        # Without this, attention's first matmuls stall
                        # ~12us on the PSUM banks held by the RoPE queue,
                        # and the idle PE re-triggers the HAM half-clock.
                        xt2 = []
                        for ksi, ks in enumerate((2, 3)):
                            xt_c2 = xt_pool.tile([128, 8, 512], BF16,
                                                 tag="xt", name=f"xt2_{ksi}")
                            nc.sync.dma_start(
                                xt_c2[:],
                                xT[:, ks * 8:(ks + 1) * 8, c0:c0 + 512])
                            xt2.append(xt_c2)
                        for j in (HQ, 0, 1, 2, 3, HQ + 1):   # K, Qs, V
                            for ksi in range(2):
                                for k in range(8):
                                    nc.tensor.matmul(
                                        pss[j][:],
                                        wqkv[:, (ksi + 2) * 8 + k,
                                             j * HD:(j + 1) * HD],
                                        xt2[ksi][:, k, :],
                                        start=False,
                                        stop=(ksi == 1 and k == 7))
                            if j < HQ + 1:
                                rope(j, pss[j])
                            else:
                                v_sb = vtmp_pool.tile([128, 512], BF16)
                                nc.vector.tensor_copy(v_sb[:], pss[j][:])
                                tp4 = trps.tile([128, 4, 128], BF16)
                                for h2 in range(4):
                                    nc.tensor.transpose(
                                        tp4[:, h2, :],
                                        v_sb[:, h2 * 128:(h2 + 1) * 128],
                                        ident[:])
                                nc.vector.tensor_copy(v_t[tb][:], tp4[:])

                # ---------- A: attention (writes att_h in SBUF) ----------
                # One flat software pipeline across all (qb, h, kt) steps:
                # the PE issues scores(i+1..i+DEPTH) while ACT exps pT(i),
                # so neither the exp latency nor the per-head pipeline
                # refill ever stalls the in-order PE. Diagonal block j
                # trims its q-range to [j*128, 512): everything earlier is
                # fully masked (exp == 0), so neither scores nor PV/sums
                # need to touch it.
                with (
                    tc.tile_pool(name="pT", bufs=7) as p_pool,
                    tc.tile_pool(name="rcp", bufs=2) as r_pool,
                    tc.tile_pool(name="sps", bufs=4, space="PSUM") as sps,
                    tc.tile_pool(name="sums", bufs=2, space="PSUM") as sums_ps,
                    tc.tile_pool(name="ops", bufs=2, space="PSUM") as o_ps_pool,
                ):
                    DEPTH = 4
                    acc = {}
                    pend = []

                    def a_flush(n):
                        while len(pend) > n:
                            pv, qb, h, kt, q0, last = pend.pop(0)
                            sum_ps, o_ps = acc[(qb, h)]
                            nc.tensor.matmul(
                                sum_ps[:, q0:], ones_col[:], pv[:, q0:],
                                start=(kt == 0), stop=last)
                            nc.tensor.matmul(
                                o_ps[:, q0:], v_t[kt // 4][:, kt % 4, :],
                                pv[:, q0:],
                                start=(kt == 0), stop=last)
                            if last:
                                recip = r_pool.tile([1, QW], F32, tag="rcp")
                                nc.vector.reciprocal(recip[:], sum_ps[:])
                                bc_sb = r_pool.tile([128, QW], F32, tag="bc")
                                nc.gpsimd.partition_broadcast(
                                    bc_sb[:], recip[:])
                                nc.vector.tensor_mul(
                                    att_q[qb][:, h, :], o_ps[:], bc_sb[:])
                                del acc[(qb, h)]

                    for qb in range(QB):
                        nkt = 4 * (qb + 1)
                        for h in range(HQ):
                            acc[(qb, h)] = (
                                sums_ps.tile([1, QW], F32, name="sum_ps"),
                                o_ps_pool.tile([128, QW], F32, name="o_ps"))
                            for kt in range(nkt):
                                j = kt - 4 * qb
                                q0 = j * 128 if j >= 0 else 0
                                ki = kt % 4
                                s_ps = sps.tile([128, QW], F32, tag="s_ps")
                                nc.tensor.matmul(
                                    s_ps[:, q0:],
                                    kT_t[kt // 4][:, ki * 128:(ki + 1) * 128],
                                    qT_t[qb][:, h, q0:],
                                    start=True, stop=True)
                                if j >= 0:
                                    nc.vector.tensor_add(
                                        s_ps[:, q0:q0 + 128],
                                        s_ps[:, q0:q0 + 128], tri[:])
                                pT = p_pool.tile([128, QW], BF16)
                                nc.scalar.activation(
                                    pT[:, q0:], s_ps[:, q0:], EXP, bias=0.0,
                                    scale=SCALE)
                                pend.append(
                                    (pT, qb, h, kt, q0, kt == nkt - 1))
                                a_flush(DEPTH)
                    a_flush(0)

                # ---------- W: output projection partial ----------
                with (
                    tc.tile_pool(name="osb", bufs=2) as osb_pool,
                    tc.tile_pool(name="wps", bufs=5, space="PSUM") as wps,
                ):
                    for mb in range(8):          # 512-wide output columns
                        m0 = mb * 512
                        for tg in range(4):      # groups of 4 token tiles
                            g0 = b * (S // 128) + tg * 4
                            last = (mb == 7 and tg == 3)
                            o_sb = osb_pool.tile([128, 4, 512], F32)
                            for ts in range(4):
                                tt = tg * 4 + ts
                                ti = tt % 4
                                ps_w = wps.tile([128, 512], F32)
                                for d4 in range(HQ):
                                    nc.tensor.matmul(
                                        ps_w[:],
                                        att_q[tt // 4][
                                            :, d4, ti * 128:(ti + 1) * 128],
                                        wo_all[:, d4, m0:m0 + 512],
                                        start=(d4 == 0), stop=(d4 == HQ - 1))
                                nc.vector.tensor_copy(o_sb[:, ts, :], ps_w[:])
                                if last:
                                    # per-tile stores for the final group so
                                    # the end-of-kernel DMA drain is short
                                    nc.scalar.dma_start(
                                        out_v[:, g0 + ts, m0:m0 + 512],
                                        o_sb[:, ts, :])
                            if not last:
                                # stores go on the scalar queue so the sync
                                # queue can prefetch the next batch's x tiles
                                nc.scalar.dma_start(
                                    out_v[:, g0:g0 + 4, m0:m0 + 512],
                                    o_sb[:])

    nc.compile()
    return nc


def _build_general():
    """Fallback for a non-causal mask: baseline f32r kernel, full mask."""
    nc = bacc.Bacc("TRN2", target_bir_lowering=False, debug=False)

    xT_d = nc.dram_tensor("xT", [DIM, TOK], F32R, kind="ExternalInput")
    w_d = nc.dram_tensor("wqkvT", [DIM, NJ * HD], F32R, kind="ExternalInput")
    wo_d = nc.dram_tensor("woT", [HQ * HD, DIM], F32R, kind="ExternalInput")
    cos_d = nc.dram_tensor("cosT", [HD, S], F32, kind="ExternalInput")
    sin_d = nc.dram_tensor("sinTs", [HD, S], F32, kind="ExternalInput")
    mask_d = nc.dram_tensor("maskTd", [QB, KT, 128, QW], F32,
                            kind="ExternalInput")
    out_d = nc.dram_tensor("out_part", [TOK, DIM], F32, kind="ExternalOutput")

    xT = xT_d.ap().rearrange("(kt p) t -> p kt t", p=128)
    w_ap = w_d.ap().rearrange("(kt p) j -> p kt j", p=128)
    wo_ap = wo_d.ap().rearrange("(dt p) m -> p dt m", p=128)
    out_v = out_d.ap().rearrange("(g p) m -> p g m", p=128)

    with tile.TileContext(nc) as tc:
        with (
            tc.tile_pool(name="const", bufs=1) as const_pool,
            tc.tile_pool(name="batch", bufs=1) as batch_pool,
            tc.tile_pool(name="dram", bufs=2, space="DRAM") as dram_pool,
        ):
            wqkv = const_pool.tile([128, 32, NJ * HD], F32R)
            for kc in range(4):
                nc.scalar.dma_start(wqkv[:, kc * 8:(kc + 1) * 8, :],
                                    w_ap[:, kc * 8:(kc + 1) * 8, :])
            ident = const_pool.tile([128, 128], F32)
            masks.make_identity(nc, ident[:])
            ones_f = const_pool.tile([128, 1], F32)
            nc.vector.memset(ones_f[:], 1.0)
            ones_col = const_pool.tile([128, 1], F32R)
            nc.vector.tensor_copy(ones_col[:], ones_f[:])

            kT_s = batch_pool.tile([128, S], F32R)
            v_s = batch_pool.tile([128, KT, HD], F32R)
            att_h = batch_pool.tile([128, HQ, S], F32R)

            for b in range(B):
                qT_d = dram_pool.tile([HQ, HD, S], F32R)

                with (
                    tc.tile_pool(name="xt", bufs=2) as xt_pool,
                    tc.tile_pool(name="cs", bufs=2) as cs_pool,
                    tc.tile_pool(name="rope", bufs=2) as rope_pool,
                    tc.tile_pool(name="vtmp", bufs=2) as vtmp_pool,
                    tc.tile_pool(name="p1ps", bufs=NJ, space="PSUM") as p1ps,
                    tc.tile_pool(name="trps", bufs=2, space="PSUM") as trps,
                ):
                    for tb in range(4):
                        c0 = b * S + tb * 512
                        sl = slice(tb * 512, tb * 512 + 512)
                        cos_c = cs_pool.tile([HD, 512], F32, tag="cos")
                        sin_c = cs_pool.tile([HD, 512], F32, tag="sin")
                        nc.sync.dma_start(cos_c[:], cos_d.ap()[:, sl])
                        nc.sync.dma_start(sin_c[:], sin_d.ap()[:, sl])
                        pss = [p1ps.tile([128, 512], F32, tag="ps",
                                         name=f"ps{j}")
                               for j in range(NJ)]
                        for ks in range(4):
                            xt_c = xt_pool.tile([128, 8, 512], F32R, tag="xt")
                            nc.sync.dma_start(
                                xt_c[:],
                                xT[:, ks * 8:(ks + 1) * 8, c0:c0 + 512])
                            for j in range(NJ):
                                for k in range(8):
                                    nc.tensor.matmul(
                                        pss[j][:],
                                        wqkv[:, ks * 8 + k,
                                             j * HD:(j + 1) * HD],
                                        xt_c[:, k, :],
                                        start=(ks == 0 and k == 0),
                                        stop=(ks == 3 and k == 7))
                        for j in range(NJ):
                            ps = pss[j]
                            if j < HQ + 1:
                                tmp = rope_pool.tile([128, 512], F32,
                                                     tag="tmp")
                                nc.vector.tensor_mul(
                                    tmp[0:64, :], ps[64:128, :],
                                    sin_c[0:64, :])
                                nc.vector.tensor_mul(
                                    tmp[64:128, :], ps[0:64, :],
                                    sin_c[64:128, :])
                                t2 = rope_pool.tile([128, 512], F32, tag="t2")
                                nc.vector.tensor_mul(t2[:], ps[:], cos_c[:])
                                if j < HQ:
                                    rT = rope_pool.tile([128, 512], F32R,
                                                        tag="rT")
                                    nc.vector.tensor_add(rT[:], t2[:], tmp[:])
                                    nc.sync.dma_start(qT_d[j, :, sl], rT[:])
                                else:
                                    nc.vector.tensor_add(
                                        kT_s[:, sl], t2[:], tmp[:])
                            else:
                                v_sb = vtmp_pool.tile([128, 512], F32)
                                nc.vector.tensor_copy(v_sb[:], ps[:])
                                for h2 in range(4):
                                    tp = trps.tile([128, 128], F32)
                                    nc.tensor.transpose(
                                        tp[:],
                                        v_sb[:, h2 * 128:(h2 + 1) * 128],
                                        ident[:])
                                    nc.vector.tensor_copy(
                                        v_s[:, tb * 4 + h2, :], tp[:])

                with (
                    tc.tile_pool(name="mask", bufs=1) as mask_pool,
                    tc.tile_pool(name="qh", bufs=3) as q_pool,
                    tc.tile_pool(name="pT", bufs=3) as p_pool,
                    tc.tile_pool(name="rcp", bufs=2) as r_pool,
                    tc.tile_pool(name="sps", bufs=3, space="PSUM") as sps,
                    tc.tile_pool(name="sums", bufs=2, space="PSUM") as sums_ps,
                    tc.tile_pool(name="ops", bufs=3, space="PSUM") as o_ps_pool,
                ):
                    for qb in range(QB):
                        m_s = mask_pool.tile([128, KT, QW], F32)
                        nc.scalar.dma_start(
                            m_s[:],
                            mask_d.ap()[qb].rearrange("kt p q -> p kt q"))
                        for h in range(HQ):
                            qh = q_pool.tile([128, QW], F32R)
                            nc.sync.dma_start(
                                qh[:], qT_d[h, :, qb * QW:(qb + 1) * QW])
                            sum_ps = sums_ps.tile([1, QW], F32)
                            o_ps = o_ps_pool.tile([128, QW], F32)
                            prev = None
                            for kt in range(KT):
                                s_ps = sps.tile([128, QW], F32, tag="s_ps")
                                nc.tensor.matmul(
                                    s_ps[:], kT_s[:, kt * 128:(kt + 1) * 128],
                                    qh[:], start=True, stop=True)
                                nc.vector.tensor_add(
                                    s_ps[:], s_ps[:], m_s[:, kt, :])
                                pT = p_pool.tile([128, QW], F32R)
                                nc.scalar.activation(
                                    pT[:], s_ps[:], EXP, bias=0.0,
                                    scale=SCALE)
                                if prev is not None:
                                    pv, pkt = prev
                                    nc.tensor.matmul(
                                        sum_ps[:], ones_col[:], pv[:],
                                        start=(pkt == 0), stop=False)
                                    nc.tensor.matmul(
                                        o_ps[:], v_s[:, pkt, :], pv[:],
                                        start=(pkt == 0), stop=False)
                                prev = (pT, kt)
                            pv, pkt = prev
                            nc.tensor.matmul(
                                sum_ps[:], ones_col[:], pv[:],
                                start=(pkt == 0), stop=True)
                            nc.tensor.matmul(
                                o_ps[:], v_s[:, pkt, :], pv[:],
                                start=(pkt == 0), stop=True)
                            recip = r_pool.tile([1, QW], F32, tag="rcp")
                            nc.vector.reciprocal(recip[:], sum_ps[:])
                            bc_sb = r_pool.tile([128, QW], F32, tag="bc")
                            nc.gpsimd.partition_broadcast(bc_sb[:], recip[:])
                            nc.vector.tensor_mul(
                                att_h[:, h, qb * QW:(qb + 1) * QW],
                                o_ps[:], bc_sb[:])

                with (
                    tc.tile_pool(name="wo", bufs=3) as wo_pool,
                    tc.tile_pool(name="osb", bufs=2) as osb_pool,
                    tc.tile_pool(name="wps", bufs=5, space="PSUM") as wps,
                ):
                    for mb in range(8):
                        wo_t = wo_pool.tile([128, HQ, 512], F32R)
                        nc.sync.dma_start(
                            wo_t[:], wo_ap[:, :, mb * 512:(mb + 1) * 512])
                        for tg in range(4):
                            o_sb = osb_pool.tile([128, 4, 512], F32)
                            for ts in range(4):
                                tt = tg * 4 + ts
                                ps_w = wps.tile([128, 512], F32)
                                for d4 in range(HQ):
                                    nc.tensor.matmul(
                                        ps_w[:],
                                        att_h[:, d4, tt * 128:(tt + 1) * 128],
                                        wo_t[:, d4, :],
                                        start=(d4 == 0), stop=(d4 == HQ - 1))
                                nc.vector.tensor_copy(o_sb[:, ts, :], ps_w[:])
                            g0 = b * (S // 128) + tg * 4
                            nc.sync.dma_start(
                                out_v[:, g0:g0 + 4, mb * 512:(mb + 1) * 512],
                                o_sb[:])

    nc.compile()
    return nc


_CACHE = {}
LAST_EXEC_NS = None


def _get_nc(causal: bool):
    if causal not in _CACHE:
        _CACHE[causal] = _build_causal() if causal else _build_general()
    return _CACHE[causal]


def _host_prep(x, wq, wk, wv, wo, freqs_cos, freqs_sin, mask):
    perm = np.concatenate([np.arange(0, HD, 2), np.arange(1, HD, 2)])
    wq_p = wq.reshape(NH, HD, DIM)[:, perm, :].reshape(NH * HD, DIM)
    wk_p = wk.reshape(NKV, HD, DIM)[:, perm, :].reshape(NKV * HD, DIM)

    xT = np.ascontiguousarray(x.reshape(TOK, DIM).T)

    cos = freqs_cos.T                     # [64, S]
    sin = freqs_sin.T
    cosT = np.ascontiguousarray(np.concatenate([cos, cos], 0))       # [128, S]
    sinTs = np.ascontiguousarray(np.concatenate([-sin, sin], 0))

    ref_mask = np.triu(np.full((S, S), -1e9, dtype=np.float32), k=1)
    causal = np.array_equal(mask, ref_mask)

    in_maps = []
    if causal:
        xTb = xT.astype(NPBF16)
        # diagonal-block triangle in [k, q] layout: -inf where k > q
        triM = np.tril(np.full((128, 128), -1e9 / np.float32(SCALE),
                               dtype=np.float32), -1)
        triM = np.ascontiguousarray(triM)
        for c in range(NCORES):
            wqT = wq_p[c * HQ * HD:(c + 1) * HQ * HD, :].T      # [DIM, 512]
            wkT = wk_p[c * HD:(c + 1) * HD, :].T                # [DIM, 128]
            wvT = wv[c * HD:(c + 1) * HD, :].T                  # [DIM, 128]
            wqkvT = np.ascontiguousarray(
                np.concatenate([wqT, wkT, wvT], 1)).astype(NPBF16)
            woT = np.ascontiguousarray(
                wo[:, c * HQ * HD:(c + 1) * HQ * HD].T).astype(NPBF16)
            in_maps.append({
                "xT": xTb, "wqkvT": wqkvT, "woT": woT,
                "cosT": cosT, "sinTs": sinTs, "triM": triM,
            })
        return causal, in_maps

    maskT = np.ascontiguousarray(mask.T) / np.float32(SCALE)   # [k, q]
    maskTd = np.empty((QB, KT, 128, QW), dtype=np.float32)
    for qb in range(QB):
        for j in range(KT):
            maskTd[qb, j] = maskT[j * 128:(j + 1) * 128,
                                  qb * QW:(qb + 1) * QW]
    for c in range(NCORES):
        wqT = wq_p[c * HQ * HD:(c + 1) * HQ * HD, :].T
        wkT = wk_p[c * HD:(c + 1) * HD, :].T
        wvT = wv[c * HD:(c + 1) * HD, :].T
        wqkvT = np.ascontiguousarray(np.concatenate([wqT, wkT, wvT], 1))
        woT = np.ascontiguousarray(wo[:, c * HQ * HD:(c + 1) * HQ * HD].T)
        in_maps.append({
            "xT": xT, "wqkvT": wqkvT, "woT": woT,
            "cosT": cosT, "sinTs": sinTs, "maskTd": maskTd,
        })
    return causal, in_maps


def kernel(x, wq, wk, wv, wo, freqs_cos, freqs_sin, mask, start_pos):
    global LAST_EXEC_NS
    causal, in_maps = _host_prep(
        np.asarray(x, np.float32), np.asarray(wq, np.float32),
        np.asarray(wk, np.float32), np.asarray(wv, np.float32),
        np.asarray(wo, np.float32), np.asarray(freqs_cos, np.float32),
        np.asarray(freqs_sin, np.float32), np.asarray(mask, np.float32))

    nc = _get_nc(causal)
    res = run_bass_kernel_spmd(nc, in_maps, core_ids=list(range(NCORES)))
    LAST_EXEC_NS = res.exec_time_ns

    acc = res.results[0]["out_part"].astype(np.float64)
    for c in range(1, NCORES):
        acc += res.results[c]["out_part"]
    return acc.astype(np.float32).reshape(B, S, DIM)


if __name__ == "__main__":
    rng = np.random.default_rng(0)
    inputs = {
        "x": rng.standard_normal((B, S, DIM), dtype=np.float32),
        "wq": (rng.standard_normal((DIM, DIM), dtype=np.float32) * 0.02),
        "wk": (rng.standard_normal((NKV * HD, DIM), dtype=np.float32) * 0.02),
        "wv": (rng.standard_normal((NKV * HD, DIM), dtype=np.float32) * 0.02),
        "wo": (rng.standard_normal((DIM, DIM), dtype=np.float32) * 0.02),
        "freqs_cos": rng.random((S, HD // 2), dtype=np.float32),
        "freqs_sin": rng.random((S, HD // 2), dtype=np.float32),
        "mask": np.triu(np.full((S, S), -1e9, dtype=np.float32), k=1),
        "start_pos": 0,
    }
    out = kernel(**inputs)
    print("out", out.shape, out.dtype, float(np.abs(out).mean()))


# revision 30
# speedup vs baseline: 1.1040x; 1.0578x over previous
"""Tensor-parallel llama-style attention (prefill) on 8 TRN2 NeuronCores.

Sharding: tensor-parallel over heads. Core c holds q-heads [4c, 4c+4),
kv-head c, the matching rows of wq/wk/wv, and columns [512c, 512c+512) of
wo. Each core computes a full-size partial of the output projection;
partials are summed on the host (the "all-reduce after wo").

Device-side layout (causal fast path):
  - All matmul operands are bf16 (PSUM accumulation stays fp32). bf16
    enables the compiler's fast-weight-load path and avoids the fp32
    PE power-throttle (HAM drops the PE clock to 1.2 GHz under
    sustained fp32-mode matmul); tolerance is 2e-2, bf16 lands ~1e-3.
  - Activations kept transposed (feature dim on partitions): xT
    [DIM, TOK], Q^T/K^T [128, S] per head, V in token-major chunks.
  - wo ([128, 4, DIM] bf16, 32 KiB/partition) and Q^T (16 KiB/partition)
    are SBUF-resident: no DRAM spill, no W-phase weight streaming.
  - RoPE: head-dim basis permuted on the host (even first, odd second),
    turning the interleaved rotation into a half-partition swap +
    elementwise mul/add against cos/sin tables, read from PSUM.
  - Causal mask: one [128,128] triangle tile (the diagonal-block
    pattern only depends on k-q). Off-diagonal blocks below the
    diagonal need no mask; blocks above are never computed. Diagonal
    block j additionally restricts its q-range to [j*128, 512), which
    recovers 128-granular causal savings (136/160 of the block-level
    work) while keeping 512-wide moving operands.
  - Softmax: no max-subtraction (scores*scale is O(10); exp safe in
    fp32). Row sums via a ones-vector matmul on the tensor engine; the
    reciprocal is partition-broadcast by gpsimd and applied by DVE.
"""

import math
import os
import sys

sys.path.insert(0, "/opt/trn_rl_repo")

import numpy as np
import ml_dtypes

import concourse.bacc as bacc
import concourse.tile as tile
import concourse.mybir as mybir
from concourse import masks
from concourse.bass_utils import run_bass_kernel_spmd

B, S, DIM = 2, 2048, 4096
TOK = B * S
NH, NKV, HD = 32, 8, 128
NCORES = 8
HQ = NH // NCORES            # 4 query heads per core
SCALE = 1.0 / math.sqrt(HD)
F32 = mybir.dt.float32
F32R = mybir.dt.float32r
BF16 = mybir.dt.bfloat16
NPBF16 = ml_dtypes.bfloat16
EXP = mybir.ActivationFunctionType.Exp

QB = 4          # q-blocks per batch (512 queries each)
QW = S // QB    # 512
KT = S // 128   # 16 k-tiles per batch
NJ = HQ + 2     # 6 projection output tiles: 4 Q heads, K, V


def _build_causal():
    nc = bacc.Bacc("TRN2", target_bir_lowering=False, debug=False)

    xT_d = nc.dram_tensor("xT", [DIM, TOK], BF16, kind="ExternalInput")
    w_d = nc.dram_tensor("wqkvT", [DIM, NJ * HD], BF16, kind="ExternalInput")
    wo_d = nc.dram_tensor("woT", [HQ * HD, DIM], BF16, kind="ExternalInput")
    cos_d = nc.dram_tensor("cosT", [HD, S], F32, kind="ExternalInput")
    sin_d = nc.dram_tensor("sinTs", [HD, S], F32, kind="ExternalInput")
    tri_d = nc.dram_tensor("triM", [128, 128], F32, kind="ExternalInput")
    out_d = nc.dram_tensor("out_part", [TOK, DIM], F32, kind="ExternalOutput")

    xT = xT_d.ap().rearrange("(kt p) t -> p kt t", p=128)      # [128, 32, TOK]
    w_ap = w_d.ap().rearrange("(kt p) j -> p kt j", p=128)     # [128, 32, 768]
    wo_ap = wo_d.ap().rearrange("(dt p) m -> p dt m", p=128)   # [128, 4, DIM]
    out_v = out_d.ap().rearrange("(g p) m -> p g m", p=128)    # [128, 32, DIM]

    with tile.TileContext(nc) as tc:
        with (
            tc.tile_pool(name="const", bufs=1) as const_pool,
            tc.tile_pool(name="batch", bufs=1) as batch_pool,
            tc.tile_pool(name="xt", bufs=3) as xt_pool,
        ):
            wqkv = const_pool.tile([128, 32, NJ * HD], BF16)
            kc0 = 0
            for kcn in (2, 2, 4, 4, 4, 4, 4, 4, 4):  # small first chunks so
                nc.scalar.dma_start(wqkv[:, kc0:kc0 + kcn, :],  # P1 starts early
                                    w_ap[:, kc0:kc0 + kcn, :])
                kc0 += kcn
            wo_all = const_pool.tile([128, HQ, DIM], BF16)
            for mc in range(4):
                nc.gpsimd.dma_start(wo_all[:, :, mc * 1024:(mc + 1) * 1024],
                                    wo_ap[:, :, mc * 1024:(mc + 1) * 1024])
            tri = const_pool.tile([128, 128], F32)
            nc.gpsimd.dma_start(tri[:], tri_d.ap())
            ident = const_pool.tile([128, 128], BF16)
            masks.make_identity(nc, ident[:])
            ones_col = const_pool.tile([128, 1], BF16)
            nc.vector.memset(ones_col[:], 1.0)

            # per-batch SBUF-resident tensors, split per 512-token chunk:
            # dependency tracking is per-tile, so a single [128, S] tensor
            # would stall attention qb=0 on the *last* chunk's RoPE
            kT_t = [batch_pool.tile([128, 512], BF16, name=f"kT{t}")
                    for t in range(4)]
            v_t = [batch_pool.tile([128, 4, HD], BF16, name=f"v{t}")
                   for t in range(4)]
            qT_t = [batch_pool.tile([128, HQ, 512], BF16, name=f"qT{t}")
                    for t in range(4)]
            att_q = [batch_pool.tile([128, HQ, 512], BF16, name=f"att{t}")
                     for t in range(4)]

            prefetched = {}
            for b in range(B):
                # ---------- P1: QKV projections + RoPE ----------
                with (
                    tc.tile_pool(name="cs", bufs=2) as cs_pool,
                    tc.tile_pool(name="rope", bufs=2) as rope_pool,
                    tc.tile_pool(name="vtmp", bufs=2) as vtmp_pool,
                    tc.tile_pool(name="p1ps", bufs=NJ, space="PSUM") as p1ps,
                    tc.tile_pool(name="trps", bufs=2, space="PSUM") as trps,
                ):
                    for tb in range(4):          # 512-token chunks
                        c0 = b * S + tb * 512
                        sl = slice(tb * 512, tb * 512 + 512)
                        cos_c = cs_pool.tile([HD, 512], F32, tag="cos")
                        sin_c = cs_pool.tile([HD, 512], F32, tag="sin")
                        nc.gpsimd.dma_start(cos_c[:], cos_d.ap()[:, sl])
                        nc.gpsimd.dma_start(sin_c[:], sin_d.ap()[:, sl])
                        pss = [p1ps.tile([128, 512], F32, tag="ps",
                                         name=f"ps{j}")
                               for j in range(NJ)]

                        def rope(j, ps):
                            # RoPE: out = z*cos + swap64(z)*sin_signed
                            tmp = rope_pool.tile([128, 512], F32, tag="tmp")
                            nc.vector.tensor_mul(
                                tmp[0:64, :], ps[64:128, :], sin_c[0:64, :])
                            nc.vector.tensor_mul(
                                tmp[64:128, :], ps[0:64, :], sin_c[64:128, :])
                            t2 = rope_pool.tile([128, 512], F32, tag="t2")
                            nc.vector.tensor_mul(t2[:], ps[:], cos_c[:])
                            if j < HQ:
                                nc.vector.tensor_add(
                                    qT_t[tb][:, j, :], t2[:], tmp[:])
                            else:
                                nc.vector.tensor_add(
                                    kT_t[tb][:], t2[:], tmp[:])

                        # pass 1 (k slices 0..15): k-outer, all 6 outputs
                        for ks in range(2):
                            if (b, tb, ks) in prefetched:
                                xt_c = prefetched.pop((b, tb, ks))
                            else:
                                xt_c = xt_pool.tile([128, 8, 512], BF16,
                                                    tag="xt")
                                if b == 0 and tb == 0 and ks == 0:
                                    # split the very first load: compute
                                    # starts after just 2 x-tiles land
                                    nc.sync.dma_start(
                                        xt_c[:, 0:2, :],
                                        xT[:, 0:2, c0:c0 + 512])
                                    nc.sync.dma_start(
                                        xt_c[:, 2:8, :],
                                        xT[:, 2:8, c0:c0 + 512])
                                else:
                                    nc.sync.dma_start(
                                        xt_c[:],
                                        xT[:, ks * 8:(ks + 1) * 8,
                                           c0:c0 + 512])
                            for k in range(8):
                                for j in range(NJ):
                                    nc.tensor.matmul(
                                        pss[j][:],
                                        wqkv[:, ks * 8 + k,
                                             j * HD:(j + 1) * HD],
                                        xt_c[:, k, :],
                                        start=(ks == 0 and k == 0),
                                        stop=False)
                        # pass 2 (k slices 16..31): j-outer so each output
                        # finishes (and its RoPE drains its PSUM bank)
                        # while the remaining outputs are still matmuling.
                        # Without this, attention's first matmuls stall
                        # ~12us on the PSUM banks held by the RoPE queue,
                        # and the idle PE re-triggers the HAM half-clock.
                        xt2 = []
                        for ksi, ks in enumerate((2, 3)):
                            xt_c2 = xt_pool.tile([128, 8, 512], BF16,
                                                 tag="xt", name=f"xt2_{ksi}")
                            nc.sync.dma_start(
                                xt_c2[:],
                                xT[:, ks * 8:(ks + 1) * 8, c0:c0 + 512])
                            xt2.append(xt_c2)
                        # Qs first: the A phase's score-PSUM pool will get
                        # the lowest-address banks (the Q accumulators), so
                        # those must drain earliest
                        for j in (0, 1, 2, 3, HQ, HQ + 1):   # Qs, K, V
                            for ksi in range(2):
                                for k in range(8):
                                    nc.tensor.matmul(
                                        pss[j][:],
                                        wqkv[:, (ksi + 2) * 8 + k,
                                             j * HD:(j + 1) * HD],
                                        xt2[ksi][:, k, :],
                                        start=False,
                                        stop=(ksi == 1 and k == 7))
                            if j < HQ + 1:
                                rope(j, pss[j])
                            else:
                                v_sb = vtmp_pool.tile([128, 512], BF16)
                                nc.vector.tensor_copy(v_sb[:], pss[j][:])
                                tp4 = trps.tile([128, 4, 128], BF16)
                                for h2 in range(4):
                                    nc.tensor.transpose(
                                        tp4[:, h2, :],
                                        v_sb[:, h2 * 128:(h2 + 1) * 128],
                                        ident[:])
                                nc.vector.tensor_copy(v_t[tb][:], tp4[:])

                # prefetch the next batch's first x chunks NOW: these DMA
                # issues must precede the A/W-phase instructions on the
                # issuing engine's queue, or the loads only start once the
                # whole previous batch has drained
                if b + 1 < B:
                    for ks in range(2):
                        xt_p = xt_pool.tile([128, 8, 512], BF16, tag="xt",
                                            name=f"xtp{ks}")
                        nc.sync.dma_start(
                            xt_p[:],
                            xT[:, ks * 8:(ks + 1) * 8,
                               (b + 1) * S:(b + 1) * S + 512])
                        prefetched[(b + 1, 0, ks)] = xt_p

                # ---------- A: attention (writes att_h in SBUF) ----------
                # One flat software pipeline across all (qb, h, kt) steps:
                # the PE issues scores(i+1..i+DEPTH) while ACT exps pT(i),
                # so neither the exp latency nor the per-head pipeline
                # refill ever stalls the in-order PE. Diagonal block j
                # trims its q-range to [j*128, 512): everything earlier is
                # fully masked (exp == 0), so neither scores nor PV/sums
                # need to touch it.
                with (
                    tc.tile_pool(name="pT", bufs=7) as p_pool,
                    tc.tile_pool(name="rcp", bufs=2) as r_pool,
                    tc.tile_pool(name="sps", bufs=4, space="PSUM") as sps,
                    tc.tile_pool(name="sums", bufs=2, space="PSUM") as sums_ps,
                    tc.tile_pool(name="ops", bufs=2, space="PSUM") as o_ps_pool,
                ):
                    DEPTH = 4
                    acc = {}
                    pend = []

                    def a_flush(n):
                        while len(pend) > n:
                            pv, qb, h, kt, q0, last = pend.pop(0)
                            sum_ps, o_ps = acc[(qb, h)]
                            nc.tensor.matmul(
                                sum_ps[:, q0:], ones_col[:], pv[:, q0:],
                                start=(kt == 0), stop=last)
                            nc.tensor.matmul(
                                o_ps[:, q0:], v_t[kt // 4][:, kt % 4, :],
                                pv[:, q0:],
                                start=(kt == 0), stop=last)
                            if last:
                                recip = r_pool.tile([1, QW], F32, tag="rcp")
                                nc.vector.reciprocal(recip[:], sum_ps[:])
                                bc_sb = r_pool.tile([128, QW], F32, tag="bc")
                                nc.gpsimd.partition_broadcast(
                                    bc_sb[:], recip[:])
                                nc.vector.tensor_mul(
                                    att_q[qb][:, h, :], o_ps[:], bc_sb[:])
                                del acc[(qb, h)]

                    for qb in range(QB):
                        nkt = 4 * (qb + 1)
                        for h in range(HQ):
                            acc[(qb, h)] = (
                                sums_ps.tile([1, QW], F32, name="sum_ps"),
                                o_ps_pool.tile([128, QW], F32, name="o_ps"))
                            for kt in range(nkt):
                                j = kt - 4 * qb
                                q0 = j * 128 if j >= 0 else 0
                                ki = kt % 4
                                s_ps = sps.tile([128, QW], F32, tag="s_ps")
                                nc.tensor.matmul(
                                    s_ps[:, q0:],
                                    kT_t[kt // 4][:, ki * 128:(ki + 1) * 128],
                                    qT_t[qb][:, h, q0:],
                                    start=True, stop=True)
                                if j >= 0:
                                    nc.vector.tensor_add(
                                        s_ps[:, q0:q0 + 128],
                                        s_ps[:, q0:q0 + 128], tri[:])
                                pT = p_pool.tile([128, QW], BF16)
                                nc.scalar.activation(
                                    pT[:, q0:], s_ps[:, q0:], EXP, bias=0.0,
                                    scale=SCALE)
                                pend.append(
                                    (pT, qb, h, kt, q0, kt == nkt - 1))
                                a_flush(DEPTH)
                    a_flush(0)

                # ---------- W: output projection partial ----------
                with (
                    tc.tile_pool(name="osb", bufs=3) as osb_pool,
                    tc.tile_pool(name="wps", bufs=5, space="PSUM") as wps,
                ):
                    for mb in range(8):          # 512-wide output columns
                        m0 = mb * 512
                        for tg in range(4):      # groups of 4 token tiles
                            g0 = b * (S // 128) + tg * 4
                            last = (mb == 7 and tg >= 2)
                            o_sb = osb_pool.tile([128, 4, 512], F32)
                            for ts in range(4):
                                tt = tg * 4 + ts
                                ti = tt % 4
                                ps_w = wps.tile([128, 512], F32)
                                for d4 in range(HQ):
                                    nc.tensor.matmul(
                                        ps_w[:],
                                        att_q[tt // 4][
                                            :, d4, ti * 128:(ti + 1) * 128],
                                        wo_all[:, d4, m0:m0 + 512],
                                        start=(d4 == 0), stop=(d4 == HQ - 1))
                                nc.vector.tensor_copy(o_sb[:, ts, :], ps_w[:])
                                if last:
                                    # per-tile stores for the final group so
                                    # the end-of-kernel DMA drain is short
                                    nc.scalar.dma_start(
                                        out_v[:, g0 + ts, m0:m0 + 512],
                                        o_sb[:, ts, :])
                            if not last:
                                # stores go on the scalar queue so the sync
                                # queue can prefetch the next batch's x tiles
                                nc.scalar.dma_start(
                                    out_v[:, g0:g0 + 4, m0:m0 + 512],
                                    o_sb[:])

    nc.compile()
    return nc


def _build_general():
    """Fallback for a non-causal mask: baseline f32r kernel, full mask."""
    nc = bacc.Bacc("TRN2", target_bir_lowering=False, debug=False)

    xT_d = nc.dram_tensor("xT", [DIM, TOK], F32R, kind="ExternalInput")
    w_d = nc.dram_tensor("wqkvT", [DIM, NJ * HD], F32R, kind="ExternalInput")
    wo_d = nc.dram_tensor("woT", [HQ * HD, DIM], F32R, kind="ExternalInput")
    cos_d = nc.dram_tensor("cosT", [HD, S], F32, kind="ExternalInput")
    sin_d = nc.dram_tensor("sinTs", [HD, S], F32, kind="ExternalInput")
    mask_d = nc.dram_tensor("maskTd", [QB, KT, 128, QW], F32,
                            kind="ExternalInput")
    out_d = nc.dram_tensor("out_part", [TOK, DIM], F32, kind="ExternalOutput")

    xT = xT_d.ap().rearrange("(kt p) t -> p kt t", p=128)
    w_ap = w_d.ap().rearrange("(kt p) j -> p kt j", p=128)
    wo_ap = wo_d.ap().rearrange("(dt p) m -> p dt m", p=128)
    out_v = out_d.ap().rearrange("(g p) m -> p g m", p=128)

    with tile.TileContext(nc) as tc:
        with (
            tc.tile_pool(name="const", bufs=1) as const_pool,
            tc.tile_pool(name="batch", bufs=1) as batch_pool,
            tc.tile_pool(name="dram", bufs=2, space="DRAM") as dram_pool,
        ):
            wqkv = const_pool.tile([128, 32, NJ * HD], F32R)
            for kc in range(4):
                nc.scalar.dma_start(wqkv[:, kc * 8:(kc + 1) * 8, :],
                                    w_ap[:, kc * 8:(kc + 1) * 8, :])
            ident = const_pool.tile([128, 128], F32)
            masks.make_identity(nc, ident[:])
            ones_f = const_pool.tile([128, 1], F32)
            nc.vector.memset(ones_f[:], 1.0)
            ones_col = const_pool.tile([128, 1], F32R)
            nc.vector.tensor_copy(ones_col[:], ones_f[:])

            kT_s = batch_pool.tile([128, S], F32R)
            v_s = batch_pool.tile([128, KT, HD], F32R)
            att_h = batch_pool.tile([128, HQ, S], F32R)

            for b in range(B):
                qT_d = dram_pool.tile([HQ, HD, S], F32R)

                with (
                    tc.tile_pool(name="xt", bufs=2) as xt_pool,
                    tc.tile_pool(name="cs", bufs=2) as cs_pool,
                    tc.tile_pool(name="rope", bufs=2) as rope_pool,
                    tc.tile_pool(name="vtmp", bufs=2) as vtmp_pool,
                    tc.tile_pool(name="p1ps", bufs=NJ, space="PSUM") as p1ps,
                    tc.tile_pool(name="trps", bufs=2, space="PSUM") as trps,
                ):
                    for tb in range(4):
                        c0 = b * S + tb * 512
                        sl = slice(tb * 512, tb * 512 + 512)
                        cos_c = cs_pool.tile([HD, 512], F32, tag="cos")
                        sin_c = cs_pool.tile([HD, 512], F32, tag="sin")
                        nc.sync.dma_start(cos_c[:], cos_d.ap()[:, sl])
                        nc.sync.dma_start(sin_c[:], sin_d.ap()[:, sl])
                        pss = [p1ps.tile([128, 512], F32, tag="ps",
                                         name=f"ps{j}")
                               for j in range(NJ)]
                        for ks in range(4):
                            xt_c = xt_pool.tile([128, 8, 512], F32R, tag="xt")
                            nc.sync.dma_start(
                                xt_c[:],
                                xT[:, ks * 8:(ks + 1) * 8, c0:c0 + 512])
                            for j in range(NJ):
                                for k in range(8):
                                    nc.tensor.matmul(
                                        pss[j][:],
                                        wqkv[:, ks * 8 + k,
                                             j * HD:(j + 1) * HD],
                                        xt_c[:, k, :],
                                        start=(ks == 0 and k == 0),
                                        stop=(ks == 3 and k == 7))
                        for j in range(NJ):
                            ps = pss[j]
                            if j < HQ + 1:
                                tmp = rope_pool.tile([128, 512], F32,
                                                     tag="tmp")
                                nc.vector.tensor_mul(
                                    tmp[0:64, :], ps[64:128, :],
                                    sin_c[0:64, :])
                                nc.vector.tensor_mul(
                                    tmp[64:128, :], ps[0:64, :],
                                    sin_c[64:128, :])
                                t2 = rope_pool.tile([128, 512], F32, tag="t2")
                                nc.vector.tensor_mul(t2[:], ps[:], cos_c[:])
                                if j < HQ:
                                    rT = rope_pool.tile([128, 512], F32R,
                                                        tag="rT")
                                    nc.vector.tensor_add(rT[:], t2[:], tmp[:])
                                    nc.sync.dma_start(qT_d[j, :, sl], rT[:])
                                else:
                                    nc.vector.tensor_add(
                                        kT_s[:, sl], t2[:], tmp[:])
                            else:
                                v_sb = vtmp_pool.tile([128, 512], F32)
                                nc.vector.tensor_copy(v_sb[:], ps[:])
                                for h2 in range(4):
                                    tp = trps.tile([128, 128], F32)
                                    nc.tensor.transpose(
                                        tp[:],
                                        v_sb[:, h2 * 128:(h2 + 1) * 128],
                                        ident[:])
                                    nc.vector.tensor_copy(
                                        v_s[:, tb * 4 + h2, :], tp[:])

                with (
                    tc.tile_pool(name="mask", bufs=1) as mask_pool,
                    tc.tile_pool(name="qh", bufs=3) as q_pool,
                    tc.tile_pool(name="pT", bufs=3) as p_pool,
                    tc.tile_pool(name="rcp", bufs=2) as r_pool,
                    tc.tile_pool(name="sps", bufs=3, space="PSUM") as sps,
                    tc.tile_pool(name="sums", bufs=2, space="PSUM") as sums_ps,
                    tc.tile_pool(name="ops", bufs=3, space="PSUM") as o_ps_pool,
                ):
                    for qb in range(QB):
                        m_s = mask_pool.tile([128, KT, QW], F32)
                        nc.scalar.dma_start(
                            m_s[:],
                            mask_d.ap()[qb].rearrange("kt p q -> p kt q"))
                        for h in range(HQ):
                            qh = q_pool.tile([128, QW], F32R)
                            nc.sync.dma_start(
                                qh[:], qT_d[h, :, qb * QW:(qb + 1) * QW])
                            sum_ps = sums_ps.tile([1, QW], F32)
                            o_ps = o_ps_pool.tile([128, QW], F32)
                            prev = None
                            for kt in range(KT):
                                s_ps = sps.tile([128, QW], F32, tag="s_ps")
                                nc.tensor.matmul(
                                    s_ps[:], kT_s[:, kt * 128:(kt + 1) * 128],
                                    qh[:], start=True, stop=True)
                                nc.vector.tensor_add(
                                    s_ps[:], s_ps[:], m_s[:, kt, :])
                                pT = p_pool.tile([128, QW], F32R)
                                nc.scalar.activation(
                                    pT[:], s_ps[:], EXP, bias=0.0,
                                    scale=SCALE)
                                if prev is not None:
                                    pv, pkt = prev
                                    nc.tensor.matmul(
                                        sum_ps[:], ones_col[:], pv[:],
                                        start=(pkt == 0), stop=False)
                                    nc.tensor.matmul(
                                        o_ps[:], v_s[:, pkt, :], pv[:],
                                        start=(pkt == 0), stop=False)
                                prev = (pT, kt)
                            pv, pkt = prev
                            nc.tensor.matmul(
                                sum_ps[:], ones_col[:], pv[:],
                                start=(pkt == 0), stop=True)
                            nc.tensor.matmul(
                                o_ps[:], v_s[:, pkt, :], pv[:],
                                start=(pkt == 0), stop=True)
                            recip = r_pool.tile([1, QW], F32, tag="rcp")
                            nc.vector.reciprocal(recip[:], sum_ps[:])
                            bc_sb = r_pool.tile([128, QW], F32, tag="bc")
                            nc.gpsimd.partition_broadcast(bc_sb[:], recip[:])
                            nc.vector.tensor_mul(
                                att_h[:, h, qb * QW:(qb + 1) * QW],
                                o_ps[:], bc_sb[:])

                with (
                    tc.tile_pool(name="wo", bufs=3) as wo_pool,
                    tc.tile_pool(name="osb", bufs=2) as osb_pool,
                    tc.tile_pool(name="wps", bufs=5, space="PSUM") as wps,
                ):
                    for mb in range(8):
                        wo_t = wo_pool.tile([128, HQ, 512], F32R)
                        nc.sync.dma_start(
                            wo_t[:], wo_ap[:, :, mb * 512:(mb + 1) * 512])
                        for tg in range(4):
                            o_sb = osb_pool.tile([128, 4, 512], F32)
                            for ts in range(4):
                                tt = tg * 4 + ts
                                ps_w = wps.tile([128, 512], F32)
                                for d4 in range(HQ):
                                    nc.tensor.matmul(
                                        ps_w[:],
                                        att_h[:, d4, tt * 128:(tt + 1) * 128],
                                        wo_t[:, d4, :],
                                        start=(d4 == 0), stop=(d4 == HQ - 1))
                                nc.vector.tensor_copy(o_sb[:, ts, :], ps_w[:])
                            g0 = b * (S // 128) + tg * 4
                            nc.sync.dma_start(
                                out_v[:, g0:g0 + 4, mb * 512:(mb + 1) * 512],
                                o_sb[:])

    nc.compile()
    return nc


_CACHE = {}
LAST_EXEC_NS = None


def _get_nc(causal: bool):
    if causal not in _CACHE:
        _CACHE[causal] = _build_causal() if causal else _build_general()
    return _CACHE[causal]


def _host_prep(x, wq, wk, wv, wo, freqs_cos, freqs_sin, mask):
    perm = np.concatenate([np.arange(0, HD, 2), np.arange(1, HD, 2)])
    wq_p = wq.reshape(NH, HD, DIM)[:, perm, :].reshape(NH * HD, DIM)
    wk_p = wk.reshape(NKV, HD, DIM)[:, perm, :].reshape(NKV * HD, DIM)

    xT = np.ascontiguousarray(x.reshape(TOK, DIM).T)

    cos = freqs_cos.T                     # [64, S]
    sin = freqs_sin.T
    cosT = np.ascontiguousarray(np.concatenate([cos, cos], 0))       # [128, S]
    sinTs = np.ascontiguousarray(np.concatenate([-sin, sin], 0))

    ref_mask = np.triu(np.full((S, S), -1e9, dtype=np.float32), k=1)
    causal = np.array_equal(mask, ref_mask)

    in_maps = []
    if causal:
        xTb = xT.astype(NPBF16)
        # diagonal-block triangle in [k, q] layout: -inf where k > q
        triM = np.tril(np.full((128, 128), -1e9 / np.float32(SCALE),
                               dtype=np.float32), -1)
        triM = np.ascontiguousarray(triM)
        for c in range(NCORES):
            wqT = wq_p[c * HQ * HD:(c + 1) * HQ * HD, :].T      # [DIM, 512]
            wkT = wk_p[c * HD:(c + 1) * HD, :].T                # [DIM, 128]
            wvT = wv[c * HD:(c + 1) * HD, :].T                  # [DIM, 128]
            wqkvT = np.ascontiguousarray(
                np.concatenate([wqT, wkT, wvT], 1)).astype(NPBF16)
            woT = np.ascontiguousarray(
                wo[:, c * HQ * HD:(c + 1) * HQ * HD].T).astype(NPBF16)
            in_maps.append({
                "xT": xTb, "wqkvT": wqkvT, "woT": woT,
                "cosT": cosT, "sinTs": sinTs, "triM": triM,
            })
        return causal, in_maps

    maskT = np.ascontiguousarray(mask.T) / np.float32(SCALE)   # [k, q]
    maskTd = np.empty((QB, KT, 128, QW), dtype=np.float32)
    for qb in range(QB):
        for j in range(KT):
            maskTd[qb, j] = maskT[j * 128:(j + 1) * 128,
                                  qb * QW:(qb + 1) * QW]
    for c in range(NCORES):
        wqT = wq_p[c * HQ * HD:(c + 1) * HQ * HD, :].T
        wkT = wk_p[c * HD:(c + 1) * HD, :].T
        wvT = wv[c * HD:(c + 1) * HD, :].T
        wqkvT = np.ascontiguousarray(np.concatenate([wqT, wkT, wvT], 1))
        woT = np.ascontiguousarray(wo[:, c * HQ * HD:(c + 1) * HQ * HD].T)
        in_maps.append({
            "xT": xT, "wqkvT": wqkvT, "woT": woT,
            "cosT": cosT, "sinTs": sinTs, "maskTd": maskTd,
        })
    return causal, in_maps


def kernel(x, wq, wk, wv, wo, freqs_cos, freqs_sin, mask, start_pos):
    global LAST_EXEC_NS
    causal, in_maps = _host_prep(
        np.asarray(x, np.float32), np.asarray(wq, np.float32),
        np.asarray(wk, np.float32), np.asarray(wv, np.float32),
        np.asarray(wo, np.float32), np.asarray(freqs_cos, np.float32),
        np.asarray(freqs_sin, np.float32), np.asarray(mask, np.float32))

    nc = _get_nc(causal)
    res = run_bass_kernel_spmd(nc, in_maps, core_ids=list(range(NCORES)))
    LAST_EXEC_NS = res.exec_time_ns

    acc = res.results[0]["out_part"].astype(np.float64)
    for c in range(1, NCORES):
        acc += res.results[c]["out_part"]
    return acc.astype(np.float32).reshape(B, S, DIM)


if __name__ == "__main__":
    rng = np.random.default_rng(0)
    inputs = {
        "x": rng.standard_normal((B, S, DIM), dtype=np.float32),
        "wq": (rng.standard_normal((DIM, DIM), dtype=np.float32) * 0.02),
        "wk": (rng.standard_normal((NKV * HD, DIM), dtype=np.float32) * 0.02),
        "wv": (rng.standard_normal((NKV * HD, DIM), dtype=np.float32) * 0.02),
        "wo": (rng.standard_normal((DIM, DIM), dtype=np.float32) * 0.02),
        "freqs_cos": rng.random((S, HD // 2), dtype=np.float32),
        "freqs_sin": rng.random((S, HD // 2), dtype=np.float32),
        "mask": np.triu(np.full((S, S), -1e9, dtype=np.float32), k=1),
        "start_pos": 0,
    }
    out = kernel(**inputs)
    print("out", out.shape, out.dtype, float(np.abs(out).mean()))
